# revision 124
# baseline (speedup 1.0000x reference)
"""Trainium2 Bass kernel for sparse channel-attention (XCA-style) module.

Reference computation (b=4, c=192, h=w=128, heads=6, C=32):
  qkv  = dwconv3x3(conv1x1(x, w_qkv), w_dw); ref_qkv likewise (shared weights)
  q = qkv[:, :c] (from x), k = ref_qkv[:, c:2c], v = ref_qkv[:, 2c:]
  q,k L2-normalized along tokens; attn = (q @ k^T) * temperature  [b,6,32,32]
  out = sum_i attn_w[i] * softmax(topk-threshold(attn, k_i)) @ v;  proj conv1x1.

Sharding: 8 cores = (batch 0..3) x (spatial half 0..1, 64 rows + halo).
Cross-core traffic: one 26KB AllReduce per core pair (q/k norms + q@k^T).

Device algorithm per core:
  - conv1x1 via float32r matmuls (1024-px double-buffered input granules;
    first block prefetched ahead of the weight loads)
  - dwconv3x3 q,k: chunks 0,1 on TensorE as 9 PSUM-accumulated
    diag-matmuls, chunk 2 on VectorE stt chains (engine balance)
  - dwconv3x3 v: v1 (64ch) packed two 4-row granules per K=128 block-diag
    matmul via a partition-duplicated, 4-row-shifted zv1 copy; v0 split
    PE-early / DVE-late / PE-collective-window to level the engines
  - PE transposes q,k to token-major; Gram per head-pair accumulates
    directly in PSUM across all 16 chunks (bank-aligned slices — two
    accumulation regions must never share a PSUM bank)
  - AllGather(pair) of stats in a 33-stride row layout (attn|qsq per row)
    so the consumer needs 3 DMAs + 3 adds; softmax with top-k via rank
    counting, all 4 branches batched into single wide DVE ops
  - final = (w_proj @ A_blockdiag) @ v with fo staged bf16 and out DMAd
    bf16 (host converts back to f32); ACT sqrt table preloaded at start
    so the tail pays only the exp-table load
"""

from contextlib import ExitStack

import numpy as np
import ml_dtypes

import concourse.bass as bass
import concourse.mybir as mybir
import concourse.tile as tile
from concourse import bacc
from concourse.bass_utils import run_bass_kernel_spmd

F32 = mybir.dt.float32
F32R = mybir.dt.float32r
BF16 = mybir.dt.bfloat16
AL = mybir.AluOpType
AF = mybir.ActivationFunctionType

B, CDIM, H, W = 4, 192, 128, 128
HEADS, CH = 6, 32
HB = 64                      # rows per core (half image)
ROWS = HB + 2                # halo rows in z buffer (66)
ZSTRIDE = 130                # padded row stride in z (128 + 2 zero pad cols)
ZBASE = 2                    # leading guard elements in z tiles
ZLEN = ZBASE + ROWS * ZSTRIDE + 2   # 8584
NPX = HB * W                 # output pixels per core (8192)
NIN = ROWS * W               # conv input pixels per core (8448)
KS_LIST = [16, 21, 24, 25]   # top-k values for C=32
# tap order: dw=0 taps first (even parity for DVE 2x mode)
TAPS9 = [(-1, 0), (0, 0), (1, 0), (-1, -1), (-1, 1), (0, -1), (0, 1), (1, -1), (1, 1)]

_CACHE = {}
import os
TRUNC = int(os.environ.get("KTRUNC", "9"))


def _build():
    nc = bacc.Bacc("TRN2", num_devices=8, num_swdge_queues=4)

    # ---------------- kernel I/O ----------------
    x_d = nc.dram_tensor("x_sh", [CDIM, NIN], F32R, kind="ExternalInput")
    r_d = nc.dram_tensor("ref_sh", [CDIM, NIN], F32R, kind="ExternalInput")
    wq_d = nc.dram_tensor("wq_t", [CDIM, 192], F32R, kind="ExternalInput")
    wkv_d = nc.dram_tensor("wkv_t", [CDIM, 384], F32R, kind="ExternalInput")
    dqk_d = nc.dram_tensor("dqk", [3, 128, 9 * 128], F32R, kind="ExternalInput")
    vw_d = nc.dram_tensor("vw", [CDIM, 9], F32, kind="ExternalInput")
    dv1_d = nc.dram_tensor("dv1", [128, 9 * 128], BF16, kind="ExternalInput")
    qkw_d = nc.dram_tensor("qkw", [3, 128, 9], F32, kind="ExternalInput")
    dv0_d = nc.dram_tensor("dv0", [128, 9 * 128], BF16, kind="ExternalInput")
    wp_d = nc.dram_tensor("wp_t", [CDIM, 192], BF16, kind="ExternalInput")
    temp_d = nc.dram_tensor("temp_rep", [CDIM, 1], F32, kind="ExternalInput")
    aw_d = nc.dram_tensor("aw_rep", [CDIM, 4], F32, kind="ExternalInput")
    ksr_d = nc.dram_tensor("ks_rep", [CDIM, 4], F32, kind="ExternalInput")
    idf_d = nc.dram_tensor("ident_f32", [128, 128], F32, kind="ExternalInput")
    e0_d = nc.dram_tensor("e_rep0", [HEADS, 128], F32, kind="ExternalInput")
    e1_d = nc.dram_tensor("e_rep1", [HEADS, 64], F32, kind="ExternalInput")
    out_d = nc.dram_tensor("out", [CDIM, NPX], BF16, kind="ExternalOutput")

    with tile.TileContext(nc) as tc, ExitStack() as ctx:
        consts = ctx.enter_context(tc.tile_pool(name="consts", bufs=1))
        zpool = ctx.enter_context(tc.tile_pool(name="zpool", bufs=1))
        zscp = ctx.enter_context(tc.tile_pool(name="zscp", bufs=2))
        ing = ctx.enter_context(tc.tile_pool(name="ing", bufs=2))    # input granules
        gcm = ctx.enter_context(tc.tile_pool(name="gcm", bufs=2))    # qk chan-major granules
        qktp = ctx.enter_context(tc.tile_pool(name="qktp", bufs=6))  # token-major qk tiles
        small = ctx.enter_context(tc.tile_pool(name="small", bufs=1))
        mps = ctx.enter_context(tc.tile_pool(name="mps", bufs=3, space="PSUM"))
        tps = ctx.enter_context(tc.tile_pool(name="tps", bufs=2, space="PSUM"))
        gaccp = ctx.enter_context(tc.tile_pool(name="gaccp", bufs=1, space="PSUM"))
        dram = ctx.enter_context(tc.tile_pool(name="dram", bufs=1, space="DRAM"))

        # ---------------- constant loads ----------------
        # prefetch the first 8-row input block BEFORE the weight loads so
        # the HWDGE delivers it first and the PE can start ASAP
        pref = {}
        for nm, dsrc, pw_ in (("xg0", x_d[0:128, 0:1024], 128),
                              ("xg1", x_d[128:192, 0:1024], 64),
                              ("rg0", r_d[0:128, 0:1024], 128),
                              ("rg1", r_d[128:192, 0:1024], 64)):
            t = ing.tile([pw_, 1024], F32R, tag=nm, name=nm)
            nc.sync.dma_start(t[:], dsrc)
            pref[nm] = t
        wq_sb0 = consts.tile([128, 192], F32R)
        wq_sb1 = consts.tile([64, 192], F32R)
        wkv_sb0 = consts.tile([128, 384], F32R)
        wkv_sb1 = consts.tile([64, 384], F32R)
        # weight loads ride the ACT-triggered queue so the input-granule
        # streaming DMAs (SP queue) reach the HWDGE first
        nc.scalar.dma_start(wq_sb0[:], wq_d[0:128, :])
        nc.scalar.dma_start(wq_sb1[:], wq_d[128:192, :])
        nc.scalar.dma_start(wkv_sb0[:], wkv_d[0:128, :])
        nc.scalar.dma_start(wkv_sb1[:], wkv_d[128:192, :])
        # group A: needed from the first pcc (dwconv + transposes)
        late_loads = []
        # group B: needed only from the collective window onward
        tail_loads = []
        dqk_sb = []
        for c in range(3):
            t = consts.tile([128, 9 * 128], F32R, name=f"dqk_sb{c}")
            late_loads.append((t, dqk_d[c]))
            dqk_sb.append(t)
        vw0 = consts.tile([128, 9], F32)
        vw1 = consts.tile([64, 9], F32)
        dv1_sb = consts.tile([128, 9 * 128], BF16)
        tail_loads.append((dv1_sb, dv1_d[:]))
        dv0_sb = consts.tile([128, 9 * 128], BF16)
        late_loads.append((dv0_sb, dv0_d[:]))
        qkw_sb = []
        for c in range(3):
            t = consts.tile([128, 9], F32, name=f"qkw_sb{c}")
            late_loads.append((t, qkw_d[c]))
            qkw_sb.append(t)
        late_loads.append((vw0, vw_d[0:128, :]))
        late_loads.append((vw1, vw_d[128:192, :]))
        wp0 = consts.tile([128, 192], BF16)
        wp1 = consts.tile([64, 192], BF16)
        tail_loads.append((wp0, wp_d[0:128, :]))
        tail_loads.append((wp1, wp_d[128:192, :]))
        temp0 = consts.tile([128, 1], F32)
        temp1 = consts.tile([64, 1], F32)
        tail_loads.append((temp0, temp_d[0:128, :]))
        tail_loads.append((temp1, temp_d[128:192, :]))
        aw0 = consts.tile([128, 4], F32)
        aw1 = consts.tile([64, 4], F32)
        tail_loads.append((aw0, aw_d[0:128, :]))
        tail_loads.append((aw1, aw_d[128:192, :]))
        ksr0 = consts.tile([128, 4], F32)
        ksr1 = consts.tile([64, 4], F32)
        tail_loads.append((ksr0, ksr_d[0:128, :]))
        tail_loads.append((ksr1, ksr_d[128:192, :]))
        ident_f32 = consts.tile([128, 128], F32)
        ident_r = consts.tile([128, 128], F32R)
        tail_loads.append((ident_f32, idf_d[:]))
        late_loads.append((ident_r, idf_d[:].bitcast(F32R)))
        erep0 = consts.tile([HEADS, 128], F32)
        erep1 = consts.tile([HEADS, 64], F32)
        tail_loads.append((erep0, e0_d[:]))
        tail_loads.append((erep1, e1_d[:]))

        # ---------------- z buffers ----------------
        # q,k conv outputs (z) kept in f32 (bf16 z flips top-k ranks and blows
        # the error budget), held as rolling 16-row super-chunks to fit SBUF.
        # v z-buffer stays full-size bf16 (v precision barely matters).
        SC_OUT = 16
        SC_IN = SC_OUT + 2
        ZSCLEN = ZBASE + SC_IN * ZSTRIDE + 2
        # zv1 is [128, ...]: partitions 0:64 hold v-channels 128:192 for z-row
        # r at slot r; partitions 64:128 hold the SAME channels for z-row r+4
        # at slot r (a DMA-duplicated, 4-row-shifted copy). This lets the v1
        # dwconv run as full K=128/M=128 block-diag matmuls covering two
        # 4-row granules at once.
        zv0 = zpool.tile([128, ZLEN], BF16)
        zv1 = zpool.tile([128, ZLEN], BF16)
        v0 = zpool.tile([128, NPX], BF16)
        # v1p: packed v1 output [128, NPX/2]: partitions 0:64 = 4-row groups
        # 0,2,4,..., partitions 64:128 = groups 1,3,5,...
        v1p = zpool.tile([128, NPX // 2], BF16)
        for zt in (zv0, zv1):
            nc.gpsimd.memset(zt[:, 0:ZBASE], 0.0)
            pad = zt[:, ZBASE:ZBASE + ROWS * ZSTRIDE].rearrange(
                "p (h w) -> p h w", w=ZSTRIDE)[:, :, 128:130]
            nc.gpsimd.memset(pad, 0.0)

        # Touch Sqrt once so ACT's initial function table is
        # "sqrt_and_friends" (which also holds Copy) — the tail's sqrt then
        # needs no table reload in the post-collective critical path.
        warm = small.tile([1, 2], F32, name="warm")
        nc.vector.memset(warm[:], 1.0)
        nc.scalar.sqrt(warm[:, 0:1], warm[:, 1:2])

        ncopy = [0]

        def copy_any(dst, src):
            # spread copy load: ACT takes 3 of 4 (DVE carries the c2 + v0
            # dwconv chains during the main phase)
            use_dve = (ncopy[0] % 4 == 0)
            ncopy[0] += 1
            if use_dve:
                nc.vector.tensor_copy(dst, src)
            else:
                nc.scalar.copy(dst, src)

        def zdst(zt, j0, nrows, p0, pw):
            # strided view of z rows j0..j0+nrows (cols 0..127)
            v = zt[p0:p0 + pw, ZBASE + ZSTRIDE * j0: ZBASE + ZSTRIDE * (j0 + nrows)]
            return v.rearrange("p (h w) -> p h w", w=ZSTRIDE)[:, :, 0:128]

        def ztap(zt, h0, nrows, dh, dw):
            # read view for output rows h0..h0+nrows, tap (dh, dw)
            start = ZBASE + ZSTRIDE * (h0 + 1 + dh) + dw
            v = zt[:, start:start + ZSTRIDE * nrows]
            return v.rearrange("p (h w) -> p h w", w=ZSTRIDE)[:, :, 0:128]

        # G accumulates directly in PSUM across all 16 pccs (64-matmul
        # accumulation chains). Each head-pair chunk's 256-wide slice gets
        # its OWN bank (512-stride) — two concurrent accumulation regions
        # sharing a bank corrupt each other.
        gacc = gaccp.tile([128, 1536], F32, name="gacc")
        PC_ROWS = 4

        def emit_v1_packed(h0g):
            # two 4-row granules (h0g, h0g+4) in one K=128 block-diag matmul
            # chain; partitions 64:128 of zv1 hold the 4-row-shifted dup.
            ps = mps.tile([128, 512], F32, tag="main", name="v1_ps")
            for t, (dh, dw) in enumerate(TAPS9):
                nc.tensor.matmul(
                    ps[:, :].rearrange("p (h w) -> p h w", w=W),
                    lhsT=dv1_sb[:, t * 128:(t + 1) * 128],
                    rhs=ztap(zv1, h0g, PC_ROWS, dh, dw),
                    start=(t == 0), stop=(t == 8))
            g = h0g // 8
            copy_any(v1p[:, g * 512:(g + 1) * 512], ps[:, :])

        def emit_v0_dve(h0, nrows):
            # v channels 0:128 on DVE (16-row chunks amortize the op init;
            # gpsimd ucode has no TensorScalarPtr so Pool can't take these)
            outv = v0[:, h0 * W:(h0 + nrows) * W].rearrange(
                "p (h w) -> p h w", w=W)
            for t, (dh, dw) in enumerate(TAPS9):
                iv = ztap(zv0, h0, nrows, dh, dw)
                if t == 0:
                    nc.vector.tensor_scalar(
                        out=outv, in0=iv, scalar1=vw0[:, 0:1],
                        scalar2=None, op0=AL.mult)
                else:
                    nc.vector.scalar_tensor_tensor(
                        out=outv, in0=iv, scalar=vw0[:, t:t + 1],
                        in1=outv, op0=AL.mult, op1=AL.add)

        def emit_v0_pe(h0g):
            # 4-row granule on PE diag-matmuls (fills the collective window)
            ps = mps.tile([128, 512], F32, tag="main", name="v0_ps")
            for t, (dh, dw) in enumerate(TAPS9):
                nc.tensor.matmul(
                    ps[:, :].rearrange("p (h w) -> p h w", w=W),
                    lhsT=dv0_sb[:, t * 128:(t + 1) * 128],
                    rhs=ztap(zv0, h0g, PC_ROWS, dh, dw),
                    start=(t == 0), stop=(t == 8))
            copy_any(v0[:, h0g * W:(h0g + PC_ROWS) * W], ps[:, :])

        for sc in range(4):
            # --- conv1x1 (f32r) for this super-chunk: 18 input rows ---
            zsc = []
            for c in range(3):
                t_ = zscp.tile([128, ZSCLEN], F32R, tag=f"zsc{c}", name=f"zsc{c}")
                nc.vector.memset(t_[:, 0:ZBASE].bitcast(F32), 0.0)
                padv = t_[:, ZBASE:ZBASE + SC_IN * ZSTRIDE].rearrange(
                    "p (h w) -> p h w", w=ZSTRIDE)[:, :, 128:130].bitcast(F32)
                nc.vector.memset(padv, 0.0)
                zsc.append(t_)
            for (jd, drows) in ((0, 8), (8, 8), (16, 2)):
                nd = (SC_OUT * sc + jd) * W
                dpix = drows * W
                if jd == 0 and "xg0" in pref:
                    # first block: either the startup prefetch or the carry
                    # from the previous sc's boundary load
                    xg0, xg1 = pref.pop("xg0"), pref.pop("xg1")
                    rg0, rg1 = pref.pop("rg0"), pref.pop("rg1")
                elif jd == 16 and sc < 3:
                    # the 2-row tail IS the first 2 rows of the next sc's
                    # first block: load that block now and consume its head
                    for nm, src_d, pw_ in (("xg0", x_d, 128), ("xg1", x_d, 64),
                                           ("rg0", r_d, 128), ("rg1", r_d, 64)):
                        p0 = 0 if pw_ == 128 else 128
                        t = ing.tile([pw_, 1024], F32R, tag=nm, name=nm)
                        nc.sync.dma_start(
                            t[:], src_d[p0:p0 + pw_, nd:nd + 1024])
                        pref[nm] = t
                    xg0, xg1 = pref["xg0"], pref["xg1"]
                    rg0, rg1 = pref["rg0"], pref["rg1"]
                else:
                    xg0 = ing.tile([128, 1024], F32R, tag="xg0", name="xg0")
                    xg1 = ing.tile([64, 1024], F32R, tag="xg1", name="xg1")
                    rg0 = ing.tile([128, 1024], F32R, tag="rg0", name="rg0")
                    rg1 = ing.tile([64, 1024], F32R, tag="rg1", name="rg1")
                    nc.sync.dma_start(xg0[:, 0:dpix], x_d[0:128, nd:nd + dpix])
                    nc.sync.dma_start(xg1[:, 0:dpix], x_d[128:192, nd:nd + dpix])
                    nc.sync.dma_start(rg0[:, 0:dpix], r_d[0:128, nd:nd + dpix])
                    nc.sync.dma_start(rg1[:, 0:dpix], r_d[128:192, nd:nd + dpix])
                for js in range(0, drows, 4):
                    j0 = jd + js
                    nrows = min(4, drows - js)
                    npix = nrows * W
                    o0 = js * W
                    xrow = SC_OUT * sc + j0
                    for (co0, cow, zi, p0) in ((0, 128, 0, 0), (128, 64, 1, 0)):
                        ps = mps.tile([128, 512], F32, tag="main", name="cv_ps")
                        nc.tensor.matmul(ps[0:cow, 0:npix],
                                         lhsT=wq_sb0[:, co0:co0 + cow],
                                         rhs=xg0[:, o0:o0 + npix],
                                         start=True, stop=False)
                        nc.tensor.matmul(ps[0:cow, 0:npix],
                                         lhsT=wq_sb1[:, co0:co0 + cow],
                                         rhs=xg1[:, o0:o0 + npix],
                                         start=False, stop=True)
                        src = ps[0:cow, 0:npix].rearrange("p (h w) -> p h w", w=W)
                        copy_any(zdst(zsc[zi], j0, nrows, p0, cow), src)
                    kv_tiles = ((0, 64, ("sc", 1, 64)), (64, 128, ("sc", 2, 0)),
                                (192, 128, ("v", zv0, 0)), (320, 64, ("v", zv1, 0)))
                    for (co0, cow, dst) in kv_tiles:
                        ps = mps.tile([128, 512], F32, tag="main", name="cv_ps")
                        nc.tensor.matmul(ps[0:cow, 0:npix],
                                         lhsT=wkv_sb0[:, co0:co0 + cow],
                                         rhs=rg0[:, o0:o0 + npix],
                                         start=True, stop=False)
                        nc.tensor.matmul(ps[0:cow, 0:npix],
                                         lhsT=wkv_sb1[:, co0:co0 + cow],
                                         rhs=rg1[:, o0:o0 + npix],
                                         start=False, stop=True)
                        src = ps[0:cow, 0:npix].rearrange("p (h w) -> p h w", w=W)
                        if dst[0] == "sc":
                            copy_any(zdst(zsc[dst[1]], j0, nrows, dst[2], cow), src)
                        else:
                            copy_any(zdst(dst[1], xrow, nrows, dst[2], cow), src)

            if sc == 0:
                for (tile_, dsrc) in late_loads:
                    nc.scalar.dma_start(tile_[:], dsrc)
            if sc == 2:
                for (tile_, dsrc) in tail_loads:
                    nc.scalar.dma_start(tile_[:], dsrc)
            # duplicate this sc's zv1 rows into partitions 64:128 shifted by
            # -4 rows (slot r holds z-row r+4) for the packed v1 matmuls
            r0d, r1d = (4, 18) if sc == 0 else (16 * sc + 2, 16 * sc + 18)
            nc.sync.dma_start(
                zv1[64:128, ZBASE + ZSTRIDE * (r0d - 4):ZBASE + ZSTRIDE * (r1d - 4)],
                zv1[0:64, ZBASE + ZSTRIDE * r0d:ZBASE + ZSTRIDE * r1d])
            # --- dwconv + transpose + Gram for output rows 16sc..16sc+16 ---
            for pcc in range(SC_OUT // PC_ROWS):
                h0l = pcc * PC_ROWS
                h0g = SC_OUT * sc + h0l
                grans = []
                pcg = sc * 4 + pcc
                for c in range(3):
                    g = gcm.tile([128, 512], F32R, tag=f"g{c}", name=f"gcm{c}")
                    if c == 2:
                        # DVE path: balances PE (the overall bottleneck)
                        gv = g[:].rearrange("p (h w) -> p h w", w=W)
                        for t, (dh, dw) in enumerate(TAPS9):
                            iv = ztap(zsc[c], h0l, PC_ROWS, dh, dw)
                            if t == 0:
                                nc.vector.tensor_scalar(
                                    out=gv, in0=iv, scalar1=qkw_sb[c][:, 0:1],
                                    scalar2=None, op0=AL.mult)
                            else:
                                nc.vector.scalar_tensor_tensor(
                                    out=gv, in0=iv, scalar=qkw_sb[c][:, t:t + 1],
                                    in1=gv, op0=AL.mult, op1=AL.add)
                    else:
                        ps = mps.tile([128, 512], F32, tag="main", name="dw_ps")
                        for t, (dh, dw) in enumerate(TAPS9):
                            nc.tensor.matmul(
                                ps[:, :].rearrange("p (h w) -> p h w", w=W),
                                lhsT=dqk_sb[c][:, t * 128:(t + 1) * 128],
                                rhs=ztap(zsc[c], h0l, PC_ROWS, dh, dw),
                                start=(t == 0), stop=(t == 8))
                        copy_any(g[:], ps[:])
                    grans.append(g)
                # v0 rows 0:48 ride along per-pcc: the first 6 chains on PE
                # (DVE is the bottleneck in sc0/sc1), the last 6 on DVE
                # during sc2/sc3 where DVE has slack. Rows 48:64 go to the
                # PE collective window.
                if pcg < 4:
                    emit_v0_pe(4 * pcg)
                elif 10 <= pcg < 14:
                    emit_v0_dve(16 + 4 * (pcg - 10), PC_ROWS)
                if TRUNC < 3:
                    continue
                # full 128x128 transposes only (sliced is_transpose crashes);
                # pair-grouping happens in the PSUM->SBUF copy via strided src
                qkts = []
                for r in range(PC_ROWS):
                    qkt_ps = tps.tile([128, 384], F32R, tag="tp", name="qkt_ps")
                    for c in range(3):
                        nc.tensor.transpose(
                            qkt_ps[:, 128 * c:128 * (c + 1)],
                            grans[c][:, 128 * r:128 * (r + 1)],
                            ident_r[:])
                    qkt = qktp.tile([128, 384], F32R, tag="qkt", name="qkt")
                    srcv = qkt_ps[:].rearrange(
                        "p (g hp c) -> p hp g c", g=2, hp=3)
                    copy_any(qkt[:, :].rearrange(
                        "p (hp g c) -> p hp g c", hp=3, g=2), srcv)
                    qkts.append(qkt)
                # full-width Gram (N=384 keeps f32r at 1 cyc/row); only the
                # diagonal 128-block per m-tile is accumulated
                for m in range(3):
                    # 256-wide moving window containing the pair block keeps
                    # f32r at 1 cyc/row (needs N >= 256); accumulate straight
                    # into the persistent PSUM gacc slice
                    w0 = 128 * m if m < 2 else 128
                    for r in range(PC_ROWS):
                        nc.tensor.matmul(
                            gacc[:, 512 * m:512 * m + 256],
                            lhsT=qkts[r][:, 128 * m:128 * (m + 1)],
                            rhs=qkts[r][:, w0:w0 + 256],
                            start=(pcg == 0 and r == 0),
                            stop=(pcg == 15 and r == PC_ROWS - 1))

        if TRUNC >= 4:
            # ---------------- phase 3: stats + AllReduce ----------------
            # stats layout (DMA/view-friendly, 33-stride rows):
            #   [0:4224)    128 rows x 33: q-rows 0:128 (heads 0-3): attn|qsq
            #   [4224:6336)  64 rows x 33: q-rows 128:192 (heads 4,5)
            #   [6336:6528) k-norms as (head, d)
            # AllReduce(add) over the core pair sums both spatial halves, so
            # the consumer needs no adds — just 3 loads with direct views.
            stats_in = dram.tile([6528], F32)
            stats_out = dram.tile([2, 6528], F32)
            nrm = small.tile([128, 3], F32)
            junk = small.tile([128, 128], F32)
            pstage = small.tile([32, 6 * CH], F32, name="pstage")
            for hp in range(3):
                goff = 512 * hp + (0 if hp < 2 else 128)
                gv = gacc[:, goff:goff + 128]
                nc.vector.tensor_tensor(out=junk[:], in0=gv,
                                        in1=ident_f32[:], op=AL.mult)
                nc.vector.tensor_reduce(out=nrm[:, hp:hp + 1], in_=junk[:],
                                        axis=mybir.AxisListType.X, op=AL.add)
                for j, (r0, c0) in enumerate(((0, 64), (32, 96))):
                    head = 2 * hp + j
                    nc.scalar.copy(pstage[:, CH * head:CH * (head + 1)],
                                   gv[r0:r0 + 32, c0:c0 + 32])
            # 3 producer DMAs: attn rows (all 6 heads, contiguous 33-stride),
            # qsq column (one strided view over all 3 pair-chunks), k-norms
            nc.sync.dma_start(
                stats_in[0:6336].rearrange("(h c j) -> c h j", h=6, j=33)[:, :, 0:32],
                pstage[:].rearrange("p (h d) -> p h d", d=CH))
            nc.sync.dma_start(
                stats_in[0:6336].rearrange("(c p j) -> p c j", c=3, j=33)[:, :, 32:33],
                nrm[0:64, :, None])
            nc.sync.dma_start(
                stats_in[6336:6528].rearrange("(hp p) -> p hp", hp=3),
                nrm[64:128, :])
            if TRUNC == 35:
                nc.sync.dma_start(stats_out[0], stats_in[:])
                nc.sync.dma_start(stats_out[1], stats_in[:])
            else:
                nc.gpsimd.collective_compute(
                    "AllGather", AL.bypass,
                    replica_groups=[[0, 1], [2, 3], [4, 5], [6, 7]],
                    ins=[stats_in[:].opt()], outs=[stats_out[:].opt()])
            # fill the collective window: packed v1 (8 matmul chains) and v0
            # rows 40:64 on PE (6 chains)
            for pb in range(8):
                emit_v1_packed(8 * pb)
                emit_v0_pe(32 + 4 * pb)

            if TRUNC >= 5 and TRUNC != 35:
                # ---------------- phase 4: softmax with top-k rank masks ----------------
                # one wide DMA per region pulls BOTH gathered halves; one add
                # each sums them in place
                sumA = small.tile([128, 66], F32, name="sumA")
                sumB = small.tile([64, 66], F32, name="sumB")
                ksql2 = small.tile([HEADS, 2 * CH], F32, name="ksql2")
                nc.sync.dma_start(
                    sumA[:].rearrange("p (g j) -> p g j", j=33),
                    stats_out[:, 0:4224].rearrange("g (p j) -> p g j", j=33))
                nc.sync.dma_start(
                    sumB[:].rearrange("p (g j) -> p g j", j=33),
                    stats_out[:, 4224:6336].rearrange("g (p j) -> p g j", j=33))
                nc.sync.dma_start(
                    ksql2[:].rearrange("h (g d) -> h g d", d=CH),
                    stats_out[:, 6336:6528].rearrange("g (h d) -> h g d", d=CH))
                nc.vector.tensor_tensor(out=sumA[:, 0:33], in0=sumA[:, 0:33],
                                        in1=sumA[:, 33:66], op=AL.add)
                nc.vector.tensor_tensor(out=sumB[:, 0:33], in0=sumB[:, 0:33],
                                        in1=sumB[:, 33:66], op=AL.add)
                ksql = ksql2[:, 0:CH]
                nc.vector.tensor_tensor(out=ksql, in0=ksql,
                                        in1=ksql2[:, CH:2 * CH], op=AL.add)
                attn = [sumA[:, 0:CH], sumB[:, 0:CH]]
                qsq = [sumA[:, CH:CH + 1], sumB[:, CH:CH + 1]]
                # replicate ksq across rows of each head via tiny matmul
                ksqr = [small.tile([128, CH], F32, name="ksqr0"),
                        small.tile([64, CH], F32, name="ksqr1")]
                for i, (erep, pw) in enumerate(((erep0, 128), (erep1, 64))):
                    ps = mps.tile([128, 512], F32, tag="main", name="krep_ps")
                    nc.tensor.matmul(ps[0:pw, 0:CH], lhsT=erep[:], rhs=ksql,
                                     start=True, stop=True)
                    nc.vector.tensor_copy(ksqr[i][:], ps[0:pw, 0:CH])

                abd0 = small.tile([128, 192], BF16)
                abd1 = small.tile([64, 192], BF16)
                nc.vector.memset(abd0[:], 0.0)
                nc.vector.memset(abd1[:], 0.0)

                # Two chains: ti=0 big ops on DVE, ti=1 big ops on gpsimd/Pool
                # (parallel); tiny scalar prep + reduces stay on DVE; ACT ops
                # grouped by function so the table loads (sqrt, exp) happen
                # once each.
                s_cs, iks, exs, rks, cmps = [], [], [], [], []
                for ti, pw in ((0, 128), (1, 64)):
                    tempt = temp0 if ti == 0 else temp1
                    s_c = small.tile([128, 1], F32, tag=f"s{ti}", name="s_c")
                    ik = small.tile([128, CH], F32, tag=f"ik{ti}", name="ik")
                    nc.scalar.sqrt(s_c[0:pw, :], qsq[ti])
                    nc.scalar.sqrt(ik[0:pw, :], ksqr[ti][:])
                    nc.vector.tensor_scalar(out=s_c[0:pw, :], in0=s_c[0:pw, :],
                                            scalar1=1e-12, scalar2=None, op0=AL.max)
                    nc.vector.reciprocal(s_c[0:pw, :], s_c[0:pw, :])
                    nc.vector.tensor_mul(s_c[0:pw, :], s_c[0:pw, :], tempt[:])
                    nc.vector.tensor_scalar(out=ik[0:pw, :], in0=ik[0:pw, :],
                                            scalar1=1e-12, scalar2=None, op0=AL.max)
                    nc.vector.reciprocal(ik[0:pw, :], ik[0:pw, :])
                    s_cs.append(s_c)
                    iks.append(ik)
                for ti, pw in ((0, 128), (1, 64)):
                    ve = nc.vector
                    at = attn[ti]
                    # attn = P * (temp/|q|) * (1/|k|); Pool ucode has no
                    # TensorScalarPtr, so ti=1 uses two tensor_tensor ops
                    nc.vector.scalar_tensor_tensor(
                        out=at, in0=at, scalar=s_cs[ti][0:pw, 0:1],
                        in1=iks[ti][0:pw, :], op0=AL.mult, op1=AL.mult)
                    cmp = small.tile([128, CH * CH], BF16, tag=f"cmp{ti}", name="cmp")
                    c3 = cmp[0:pw, :].rearrange("p (j k) -> p j k", k=CH)
                    ve.tensor_tensor(
                        out=c3, in0=at[:, None, :].broadcast_to([pw, CH, CH]),
                        in1=at[:, :, None].broadcast_to([pw, CH, CH]), op=AL.is_ge)
                    cmps.append(c3)
                for ti, pw in ((0, 128), (1, 64)):
                    rk = small.tile([128, CH], F32, tag=f"rk{ti}", name="rk")
                    nc.vector.tensor_reduce(out=rk[0:pw, :], in_=cmps[ti],
                                            axis=mybir.AxisListType.X, op=AL.add)
                    nmax = small.tile([128, 1], F32, tag=f"nm{ti}", name="nmax")
                    nc.vector.tensor_reduce(out=nmax[0:pw, :], in_=attn[ti],
                                            axis=mybir.AxisListType.X, op=AL.max,
                                            negate=True)
                    ex = small.tile([128, CH], F32, tag=f"ex{ti}", name="ex")
                    nc.scalar.activation(ex[0:pw, :], attn[ti], AF.Exp,
                                         bias=nmax[0:pw, 0:1], scale=1.0)
                    rks.append(rk)
                    exs.append(ex)
                for ti, pw in ((0, 128), (1, 64)):
                    ve = nc.vector
                    awt = aw0 if ti == 0 else aw1
                    ksrt = ksr0 if ti == 0 else ksr1
                    rk, ex = rks[ti], exs[ti]
                    # all 4 top-k branches batched: mk4[p, i, j] = [rk_j <= k_i]
                    mk4 = small.tile([128, 4 * CH], BF16, tag=f"mk{ti}", name="mk4")
                    m3 = mk4[0:pw, :].rearrange("p (i j) -> p i j", j=CH)
                    ve.tensor_tensor(
                        out=m3, in0=rk[0:pw, None, :].broadcast_to([pw, 4, CH]),
                        in1=ksrt[:, :, None].broadcast_to([pw, 4, CH]), op=AL.is_le)
                    junk4 = small.tile([128, 4 * CH], F32, tag=f"jk{ti}", name="junk4")
                    j3 = junk4[0:pw, :].rearrange("p (i j) -> p i j", j=CH)
                    ve.tensor_tensor(
                        out=j3, in0=m3,
                        in1=ex[0:pw, None, :].broadcast_to([pw, 4, CH]), op=AL.mult)
                    sden4 = small.tile([128, 4], F32, tag=f"sd{ti}", name="sden4")
                    nc.vector.tensor_reduce(out=sden4[0:pw, :], in_=j3,
                                            axis=mybir.AxisListType.X, op=AL.add)
                    coef4 = small.tile([128, 4], F32, tag=f"cf{ti}", name="coef4")
                    nc.vector.reciprocal(coef4[0:pw, :], sden4[0:pw, :])
                    nc.vector.tensor_mul(coef4[0:pw, :], coef4[0:pw, :],
                                         awt[:, 0:4])
                    # gt = sum_i coef4[:, i] * mk4[:, i, :] (stt is DVE-only)
                    gt = small.tile([128, CH], F32, tag=f"gt{ti}", name="gt")
                    for i in range(4):
                        if i == 0:
                            nc.vector.tensor_scalar(
                                out=gt[0:pw, :],
                                in0=mk4[0:pw, i * CH:(i + 1) * CH],
                                scalar1=coef4[0:pw, i:i + 1],
                                scalar2=None, op0=AL.mult)
                        else:
                            nc.vector.scalar_tensor_tensor(
                                out=gt[0:pw, :], in0=mk4[0:pw, i * CH:(i + 1) * CH],
                                scalar=coef4[0:pw, i:i + 1], in1=gt[0:pw, :],
                                op0=AL.mult, op1=AL.add)
                    # A blocks into block-diagonal abd (bf16)
                    abdt = abd0 if ti == 0 else abd1
                    nheads_t = 4 if ti == 0 else 2
                    for j in range(nheads_t):
                        head = j if ti == 0 else 4 + j
                        ve.tensor_tensor(
                            out=abdt[32 * j:32 * (j + 1), 32 * head:32 * (head + 1)],
                            in0=ex[32 * j:32 * (j + 1), :],
                            in1=gt[32 * j:32 * (j + 1), :], op=AL.mult)

                if TRUNC == 5:
                    nc.gpsimd.dma_start(out_d[0:128, 0:192], abd0[:])
                    nc.gpsimd.dma_start(out_d[128:192, 0:192], abd1[:])
                if TRUNC >= 9:
                    # ---------------- phase 5: M2^T = A_bd^T @ Wp^T, final = M2 @ v ----------------
                    # m2tb0: v0-part of M2^T; m2tb1: v1-part, duplicated into
                    # partitions 64:128 so odd-group v1p matmuls line up.
                    m2tb0 = small.tile([128, 192], BF16, name="m2tb0")
                    m2tb1 = small.tile([128, 192], BF16, name="m2tb1")
                    for dt_i, (d0, dw_) in enumerate(((0, 128), (128, 64))):
                        ps = mps.tile([128, 512], F32, tag="main", name="m2_ps")
                        nc.tensor.matmul(ps[0:dw_, 0:192], lhsT=abd0[:, d0:d0 + dw_],
                                         rhs=wp0[:], start=True, stop=False)
                        nc.tensor.matmul(ps[0:dw_, 0:192], lhsT=abd1[:, d0:d0 + dw_],
                                         rhs=wp1[:], start=False, stop=True)
                        if dt_i == 0:
                            nc.vector.tensor_copy(m2tb0[:], ps[0:128, 0:192])
                        else:
                            nc.vector.tensor_copy(m2tb1[0:64, :], ps[0:64, 0:192])
                            nc.vector.tensor_copy(m2tb1[64:128, :], ps[0:64, 0:192])

                    FCH = 512
                    for ct, (co0, cw) in enumerate(((0, 128), (128, 64))):
                        for fc in range(NPX // FCH):
                            n0 = fc * FCH
                            po = (fc % 2) * 64
            # rotate over 6 PSUM banks (main 4 + gp 2; gp pads to a full
                            # bank anyway) so copy-out latency doesn't
                            # throttle the matmuls
                            if fc % 3 == 2:
                                ps = tps.tile([128, 512], F32, tag="tp",
                                              name="fo_ps2")
                            else:
                                ps = mps.tile([128, 512], F32, tag="main",
                                              name="fo_ps")
                            nc.tensor.matmul(ps[0:cw, :], lhsT=m2tb0[:, co0:co0 + cw],
                                             rhs=v0[:, n0:n0 + FCH], start=True, stop=False)
                            nc.tensor.matmul(ps[0:cw, :],
                                             lhsT=m2tb1[po:po + 64, co0:co0 + cw],
                                             rhs=v1p[po:po + 64,
                                                     (fc // 2) * FCH:(fc // 2 + 1) * FCH],
                                             start=False, stop=True)
                            if fc % 2 == 0:
                                fo = ing.tile([128, 2 * FCH], BF16, tag="fo",
                                              name="fo_sb", bufs=3)
                            # alternate ACT/DVE: both are idle in the tail,
                            # and 32 serial ACT copies would gate the final
                            fdst = fo[0:cw, (fc % 2) * FCH:(fc % 2 + 1) * FCH]
                            if fc % 2 == 0:
                                nc.scalar.copy(fdst, ps[0:cw, :])
                            else:
                                nc.vector.tensor_copy(fdst, ps[0:cw, :])
                            if fc % 2 == 1:
                                nc.sync.dma_start(
                                    out_d[co0:co0 + cw, n0 - FCH:n0 + FCH],
                                    fo[0:cw, :])

    nc.finalize()
    return nc


def _prep_inputs(x, ref, w_qkv, w_dw, w_proj, temperature, attn_w):
    bf = ml_dtypes.bfloat16
    w_qkv = np.asarray(w_qkv, np.float32)[:, :, 0, 0]          # [576, 192]
    w_dw = np.asarray(w_dw, np.float32)[:, 0]                  # [576, 3, 3]
    w_proj = np.asarray(w_proj, np.float32)[:, :, 0, 0]        # [192, 192]
    temp = np.asarray(temperature, np.float32).reshape(HEADS)
    aw = np.asarray(attn_w, np.float32).reshape(4)

    wq_t = np.ascontiguousarray(w_qkv[:192].T)                 # [ci, co]
    wkv_t = np.ascontiguousarray(w_qkv[192:].T)                # [ci, 384]

    dwq, dwk, dwv = w_dw[:192], w_dw[192:384], w_dw[384:]
    chunks = [dwq[0:128], np.concatenate([dwq[128:192], dwk[0:64]]), dwk[64:192]]
    dqk = np.zeros((3, 128, 9 * 128), np.float32)
    for c, blk in enumerate(chunks):
        for t, (dh, dw_) in enumerate(TAPS9):
            np.fill_diagonal(dqk[c, :, t * 128:(t + 1) * 128], blk[:, dh + 1, dw_ + 1])
    qkw = np.zeros((3, 128, 9), np.float32)
    for c, blk in enumerate(chunks):
        for t, (dh, dw_) in enumerate(TAPS9):
            qkw[c, :, t] = blk[:, dh + 1, dw_ + 1]
    vw = np.zeros((CDIM, 9), np.float32)
    for t, (dh, dw_) in enumerate(TAPS9):
        vw[:, t] = dwv[:, dh + 1, dw_ + 1]
    dv1 = np.zeros((128, 9 * 128), np.float32)
    for t, (dh, dw_) in enumerate(TAPS9):
        np.fill_diagonal(dv1[0:64, t * 128:t * 128 + 64],
                         dwv[128:192, dh + 1, dw_ + 1])
        np.fill_diagonal(dv1[64:128, t * 128 + 64:(t + 1) * 128],
                         dwv[128:192, dh + 1, dw_ + 1])
    dv0 = np.zeros((128, 9 * 128), np.float32)
    for t, (dh, dw_) in enumerate(TAPS9):
        np.fill_diagonal(dv0[:, t * 128:(t + 1) * 128], dwv[0:128, dh + 1, dw_ + 1])

    wp_t = np.ascontiguousarray(w_proj.T)                      # [c, co]
    temp_rep = np.repeat(temp, CH).reshape(CDIM, 1)
    aw_rep = np.tile(aw[None, :], (CDIM, 1))
    ks_rep = np.tile(np.asarray(KS_LIST, np.float32)[None, :], (CDIM, 1))
    ident = np.eye(128, dtype=np.float32)
    e0 = np.zeros((HEADS, 128), np.float32)
    e1 = np.zeros((HEADS, 64), np.float32)
    for h in range(4):
        e0[h, 32 * h:32 * (h + 1)] = 1.0
    for h in range(2):
        e1[h + 4, 32 * h:32 * (h + 1)] = 1.0

    xp = np.zeros((B, CDIM, H + 2, W), np.float32)
    xp[:, :, 1:H + 1] = np.asarray(x, np.float32)
    rp = np.zeros((B, CDIM, H + 2, W), np.float32)
    rp[:, :, 1:H + 1] = np.asarray(ref, np.float32)

    common = {
        "wq_t": wq_t, "wkv_t": wkv_t,
        "dqk": dqk, "vw": vw, "qkw": qkw, "dv1": dv1.astype(bf), "dv0": dv0.astype(bf),
        "wp_t": wp_t.astype(bf), "temp_rep": temp_rep, "aw_rep": aw_rep,
        "ks_rep": ks_rep,
        "ident_f32": ident,
        "e_rep0": e0, "e_rep1": e1,
    }
    in_maps = []
    for core in range(8):
        b, s = core // 2, core % 2
        m = dict(common)
        m["x_sh"] = np.ascontiguousarray(
            xp[b, :, 64 * s:64 * s + ROWS].reshape(CDIM, NIN))
        m["ref_sh"] = np.ascontiguousarray(
            rp[b, :, 64 * s:64 * s + ROWS].reshape(CDIM, NIN))
        in_maps.append(m)
    return in_maps


def _run(inputs, trace=False):
    if "nc" not in _CACHE:
        _CACHE["nc"] = _build()
    nc = _CACHE["nc"]
    in_maps = _prep_inputs(**inputs)
    res = run_bass_kernel_spmd(nc, in_maps, core_ids=list(range(8)), trace=trace)
    out = np.zeros((B, CDIM, H, W), np.float32)
    for core in range(8):
        b, s = core // 2, core % 2
        out[b, :, 64 * s:64 * (s + 1)] = np.asarray(
            res.results[core]["out"], np.float32).reshape(CDIM, HB, W)
    return out, res


def kernel(**inputs):
    out, _ = _run(inputs, trace=False)
    return out



# revision 135
# speedup vs baseline: 1.0026x; 1.0026x over previous
"""Trainium2 Bass kernel for sparse channel-attention (XCA-style) module.

Reference computation (b=4, c=192, h=w=128, heads=6, C=32):
  qkv  = dwconv3x3(conv1x1(x, w_qkv), w_dw); ref_qkv likewise (shared weights)
  q = qkv[:, :c] (from x), k = ref_qkv[:, c:2c], v = ref_qkv[:, 2c:]
  q,k L2-normalized along tokens; attn = (q @ k^T) * temperature  [b,6,32,32]
  out = sum_i attn_w[i] * softmax(topk-threshold(attn, k_i)) @ v;  proj conv1x1.

Sharding: 8 cores = (batch 0..3) x (spatial half 0..1, 64 rows + halo).
Cross-core traffic: one 26KB AllReduce per core pair (q/k norms + q@k^T).

Device algorithm per core:
  - conv1x1 via float32r matmuls (1024-px double-buffered input granules;
    first block prefetched ahead of the weight loads)
  - dwconv3x3 q,k: chunks 0,1 on TensorE as 9 PSUM-accumulated
    diag-matmuls, chunk 2 on VectorE stt chains (engine balance)
  - dwconv3x3 v: v1 (64ch) packed two 4-row granules per K=128 block-diag
    matmul via a partition-duplicated, 4-row-shifted zv1 copy; v0 split
    PE-early / DVE-late / PE-collective-window to level the engines
  - PE transposes q,k to token-major; Gram per head-pair accumulates
    directly in PSUM across all 16 chunks (bank-aligned slices — two
    accumulation regions must never share a PSUM bank)
  - AllGather(pair) of stats in a 33-stride row layout (attn|qsq per row)
    so the consumer needs 3 DMAs + 3 adds; softmax with top-k via rank
    counting, all 4 branches batched into single wide DVE ops
  - final = (w_proj @ A_blockdiag) @ v with fo staged bf16 and out DMAd
    bf16 (host converts back to f32); ACT sqrt table preloaded at start
    so the tail pays only the exp-table load
"""

from contextlib import ExitStack

import numpy as np
import ml_dtypes

import concourse.bass as bass
import concourse.mybir as mybir
import concourse.tile as tile
from concourse import bacc
from concourse.bass_utils import run_bass_kernel_spmd

F32 = mybir.dt.float32
F32R = mybir.dt.float32r
BF16 = mybir.dt.bfloat16
AL = mybir.AluOpType
AF = mybir.ActivationFunctionType

B, CDIM, H, W = 4, 192, 128, 128
HEADS, CH = 6, 32
HB = 64                      # rows per core (half image)
ROWS = HB + 2                # halo rows in z buffer (66)
ZSTRIDE = 130                # padded row stride in z (128 + 2 zero pad cols)
ZBASE = 2                    # leading guard elements in z tiles
ZLEN = ZBASE + ROWS * ZSTRIDE + 2   # 8584
NPX = HB * W                 # output pixels per core (8192)
NIN = ROWS * W               # conv input pixels per core (8448)
KS_LIST = [16, 21, 24, 25]   # top-k values for C=32
# tap order: dw=0 taps first (even parity for DVE 2x mode)
TAPS9 = [(-1, 0), (0, 0), (1, 0), (-1, -1), (-1, 1), (0, -1), (0, 1), (1, -1), (1, 1)]

_CACHE = {}
import os
TRUNC = int(os.environ.get("KTRUNC", "9"))


def _build():
    nc = bacc.Bacc("TRN2", num_devices=8, num_swdge_queues=4)

    # ---------------- kernel I/O ----------------
    x_d = nc.dram_tensor("x_sh", [CDIM, NIN], F32R, kind="ExternalInput")
    r_d = nc.dram_tensor("ref_sh", [CDIM, NIN], F32R, kind="ExternalInput")
    wq_d = nc.dram_tensor("wq_t", [CDIM, 192], F32R, kind="ExternalInput")
    wkv_d = nc.dram_tensor("wkv_t", [CDIM, 384], F32R, kind="ExternalInput")
    dqk_d = nc.dram_tensor("dqk", [3, 128, 9 * 128], F32R, kind="ExternalInput")
    vw_d = nc.dram_tensor("vw", [CDIM, 9], F32, kind="ExternalInput")
    dv1_d = nc.dram_tensor("dv1", [128, 9 * 128], BF16, kind="ExternalInput")
    qkw_d = nc.dram_tensor("qkw", [3, 128, 9], F32, kind="ExternalInput")
    dv0_d = nc.dram_tensor("dv0", [128, 9 * 128], BF16, kind="ExternalInput")
    wp_d = nc.dram_tensor("wp_t", [CDIM, 192], BF16, kind="ExternalInput")
    temp_d = nc.dram_tensor("temp_rep", [CDIM, 1], F32, kind="ExternalInput")
    aw_d = nc.dram_tensor("aw_rep", [CDIM, 4], F32, kind="ExternalInput")
    ksr_d = nc.dram_tensor("ks_rep", [CDIM, 4], F32, kind="ExternalInput")
    idf_d = nc.dram_tensor("ident_f32", [128, 128], F32, kind="ExternalInput")
    e0_d = nc.dram_tensor("e_rep0", [HEADS, 128], F32, kind="ExternalInput")
    e1_d = nc.dram_tensor("e_rep1", [HEADS, 64], F32, kind="ExternalInput")
    out_d = nc.dram_tensor("out", [CDIM, NPX], BF16, kind="ExternalOutput")

    with tile.TileContext(nc) as tc, ExitStack() as ctx:
        consts = ctx.enter_context(tc.tile_pool(name="consts", bufs=1))
        zpool = ctx.enter_context(tc.tile_pool(name="zpool", bufs=1))
        zscp = ctx.enter_context(tc.tile_pool(name="zscp", bufs=2))
        ing = ctx.enter_context(tc.tile_pool(name="ing", bufs=2))    # input granules
        gcm = ctx.enter_context(tc.tile_pool(name="gcm", bufs=2))    # qk chan-major granules
        qktp = ctx.enter_context(tc.tile_pool(name="qktp", bufs=6))  # token-major qk tiles
        small = ctx.enter_context(tc.tile_pool(name="small", bufs=1))
        mps = ctx.enter_context(tc.tile_pool(name="mps", bufs=3, space="PSUM"))
        tps = ctx.enter_context(tc.tile_pool(name="tps", bufs=2, space="PSUM"))
        gaccp = ctx.enter_context(tc.tile_pool(name="gaccp", bufs=1, space="PSUM"))
        dram = ctx.enter_context(tc.tile_pool(name="dram", bufs=1, space="DRAM"))

        # ---------------- constant loads ----------------
        # prefetch the first 8-row input block BEFORE the weight loads so
        # the HWDGE delivers it first and the PE can start ASAP
        pref = {}
        for nm, dsrc, pw_ in (("xg0", x_d[0:128, 0:1024], 128),
                              ("xg1", x_d[128:192, 0:1024], 64),
                              ("rg0", r_d[0:128, 0:1024], 128),
                              ("rg1", r_d[128:192, 0:1024], 64)):
            t = ing.tile([pw_, 1024], F32R, tag=nm, name=nm)
            nc.sync.dma_start(t[:], dsrc)
            pref[nm] = t
        wq_sb0 = consts.tile([128, 192], F32R)
        wq_sb1 = consts.tile([64, 192], F32R)
        wkv_sb0 = consts.tile([128, 384], F32R)
        wkv_sb1 = consts.tile([64, 384], F32R)
        # weight loads ride the ACT-triggered queue so the input-granule
        # streaming DMAs (SP queue) reach the HWDGE first
        nc.scalar.dma_start(wq_sb0[:], wq_d[0:128, :])
        nc.scalar.dma_start(wq_sb1[:], wq_d[128:192, :])
        nc.scalar.dma_start(wkv_sb0[:], wkv_d[0:128, :])
        nc.scalar.dma_start(wkv_sb1[:], wkv_d[128:192, :])
        # group A: needed from the first pcc (dwconv + transposes)
        late_loads = []
        # group B: needed only from the collective window onward
        tail_loads = []
        dqk_sb = []
        for c in range(3):
            t = consts.tile([128, 9 * 128], F32R, name=f"dqk_sb{c}")
            late_loads.append((t, dqk_d[c]))
            dqk_sb.append(t)
        vw0 = consts.tile([128, 9], F32)
        vw1 = consts.tile([64, 9], F32)
        dv1_sb = consts.tile([128, 9 * 128], BF16)
        tail_loads.append((dv1_sb, dv1_d[:]))
        dv0_sb = consts.tile([128, 9 * 128], BF16)
        late_loads.append((dv0_sb, dv0_d[:]))
        qkw_sb = []
        for c in range(3):
            t = consts.tile([128, 9], F32, name=f"qkw_sb{c}")
            late_loads.append((t, qkw_d[c]))
            qkw_sb.append(t)
        late_loads.append((vw0, vw_d[0:128, :]))
        late_loads.append((vw1, vw_d[128:192, :]))
        wp0 = consts.tile([128, 192], BF16)
        wp1 = consts.tile([64, 192], BF16)
        tail_loads.append((wp0, wp_d[0:128, :]))
        tail_loads.append((wp1, wp_d[128:192, :]))
        temp0 = consts.tile([128, 1], F32)
        temp1 = consts.tile([64, 1], F32)
        tail_loads.append((temp0, temp_d[0:128, :]))
        tail_loads.append((temp1, temp_d[128:192, :]))
        aw0 = consts.tile([128, 4], F32)
        aw1 = consts.tile([64, 4], F32)
        tail_loads.append((aw0, aw_d[0:128, :]))
        tail_loads.append((aw1, aw_d[128:192, :]))
        ksr0 = consts.tile([128, 4], F32)
        ksr1 = consts.tile([64, 4], F32)
        tail_loads.append((ksr0, ksr_d[0:128, :]))
        tail_loads.append((ksr1, ksr_d[128:192, :]))
        ident_f32 = consts.tile([128, 128], F32)
        ident_r = consts.tile([128, 128], F32R)
        tail_loads.append((ident_f32, idf_d[:]))
        late_loads.append((ident_r, idf_d[:].bitcast(F32R)))
        erep0 = consts.tile([HEADS, 128], F32)
        erep1 = consts.tile([HEADS, 64], F32)
        tail_loads.append((erep0, e0_d[:]))
        tail_loads.append((erep1, e1_d[:]))

        # ---------------- z buffers ----------------
        # q,k conv outputs (z) kept in f32 (bf16 z flips top-k ranks and blows
        # the error budget), held as rolling 16-row super-chunks to fit SBUF.
        # v z-buffer stays full-size bf16 (v precision barely matters).
        SC_OUT = 16
        SC_IN = SC_OUT + 2
        ZSCLEN = ZBASE + SC_IN * ZSTRIDE + 2
        # zv1 is [128, ...]: partitions 0:64 hold v-channels 128:192 for z-row
        # r at slot r; partitions 64:128 hold the SAME channels for z-row r+4
        # at slot r (a DMA-duplicated, 4-row-shifted copy). This lets the v1
        # dwconv run as full K=128/M=128 block-diag matmuls covering two
        # 4-row granules at once.
        zv0 = zpool.tile([128, ZLEN], BF16)
        zv1 = zpool.tile([128, ZLEN], BF16)
        v0 = zpool.tile([128, NPX], BF16)
        # v1p: packed v1 output [128, NPX/2]: partitions 0:64 = 4-row groups
        # 0,2,4,..., partitions 64:128 = groups 1,3,5,...
        v1p = zpool.tile([128, NPX // 2], BF16)
        for zt in (zv0, zv1):
            nc.gpsimd.memset(zt[:, 0:ZBASE], 0.0)
            pad = zt[:, ZBASE:ZBASE + ROWS * ZSTRIDE].rearrange(
                "p (h w) -> p h w", w=ZSTRIDE)[:, :, 128:130]
            nc.gpsimd.memset(pad, 0.0)

        # Touch Sqrt once so ACT's initial function table is
        # "sqrt_and_friends" (which also holds Copy) — the tail's sqrt then
        # needs no table reload in the post-collective critical path.
        warm = small.tile([1, 2], F32, name="warm")
        nc.vector.memset(warm[:], 1.0)
        nc.scalar.sqrt(warm[:, 0:1], warm[:, 1:2])

        ncopy = [0]

        def copy_any(dst, src):
            # spread copy load: ACT takes 3 of 4 (DVE carries the c2 + v0
            # dwconv chains during the main phase)
            use_dve = (ncopy[0] % 4 == 0)
            ncopy[0] += 1
            if use_dve:
                nc.vector.tensor_copy(dst, src)
            else:
                nc.scalar.copy(dst, src)

        def zdst(zt, j0, nrows, p0, pw):
            # strided view of z rows j0..j0+nrows (cols 0..127)
            v = zt[p0:p0 + pw, ZBASE + ZSTRIDE * j0: ZBASE + ZSTRIDE * (j0 + nrows)]
            return v.rearrange("p (h w) -> p h w", w=ZSTRIDE)[:, :, 0:128]

        def ztap(zt, h0, nrows, dh, dw):
            # read view for output rows h0..h0+nrows, tap (dh, dw)
            start = ZBASE + ZSTRIDE * (h0 + 1 + dh) + dw
            v = zt[:, start:start + ZSTRIDE * nrows]
            return v.rearrange("p (h w) -> p h w", w=ZSTRIDE)[:, :, 0:128]

        # G accumulates directly in PSUM across all 16 pccs (64-matmul
        # accumulation chains). Each head-pair chunk's 256-wide slice gets
        # its OWN bank (512-stride) — two concurrent accumulation regions
        # sharing a bank corrupt each other.
        gacc = gaccp.tile([128, 1536], F32, name="gacc")
        PC_ROWS = 4

        def emit_v1_packed(h0g):
            # two 4-row granules (h0g, h0g+4) in one K=128 block-diag matmul
            # chain; partitions 64:128 of zv1 hold the 4-row-shifted dup.
            ps = tps.tile([128, 512], F32, tag="tp", name="v1_ps")
            for t, (dh, dw) in enumerate(TAPS9):
                nc.tensor.matmul(
                    ps[:, :].rearrange("p (h w) -> p h w", w=W),
                    lhsT=dv1_sb[:, t * 128:(t + 1) * 128],
                    rhs=ztap(zv1, h0g, PC_ROWS, dh, dw),
                    start=(t == 0), stop=(t == 8))
            g = h0g // 8
            copy_any(v1p[:, g * 512:(g + 1) * 512], ps[:, :])

        def emit_v0_dve(h0, nrows):
            # v channels 0:128 on DVE (16-row chunks amortize the op init;
            # gpsimd ucode has no TensorScalarPtr so Pool can't take these)
            outv = v0[:, h0 * W:(h0 + nrows) * W].rearrange(
                "p (h w) -> p h w", w=W)
            for t, (dh, dw) in enumerate(TAPS9):
                iv = ztap(zv0, h0, nrows, dh, dw)
                if t == 0:
                    nc.vector.tensor_scalar(
                        out=outv, in0=iv, scalar1=vw0[:, 0:1],
                        scalar2=None, op0=AL.mult)
                else:
                    nc.vector.scalar_tensor_tensor(
                        out=outv, in0=iv, scalar=vw0[:, t:t + 1],
                        in1=outv, op0=AL.mult, op1=AL.add)

        def emit_v0_pe(h0g):
            # 4-row granule on PE diag-matmuls (fills the collective window)
            ps = mps.tile([128, 512], F32, tag="main", name="v0_ps")
            for t, (dh, dw) in enumerate(TAPS9):
                nc.tensor.matmul(
                    ps[:, :].rearrange("p (h w) -> p h w", w=W),
                    lhsT=dv0_sb[:, t * 128:(t + 1) * 128],
                    rhs=ztap(zv0, h0g, PC_ROWS, dh, dw),
                    start=(t == 0), stop=(t == 8))
            copy_any(v0[:, h0g * W:(h0g + PC_ROWS) * W], ps[:, :])

        for sc in range(4):
            # --- conv1x1 (f32r) for this super-chunk: 18 input rows ---
            zsc = []
            for c in range(3):
                t_ = zscp.tile([128, ZSCLEN], F32R, tag=f"zsc{c}", name=f"zsc{c}")
                nc.vector.memset(t_[:, 0:ZBASE].bitcast(F32), 0.0)
                padv = t_[:, ZBASE:ZBASE + SC_IN * ZSTRIDE].rearrange(
                    "p (h w) -> p h w", w=ZSTRIDE)[:, :, 128:130].bitcast(F32)
                nc.vector.memset(padv, 0.0)
                zsc.append(t_)
            for (jd, drows) in ((0, 8), (8, 8), (16, 2)):
                nd = (SC_OUT * sc + jd) * W
                dpix = drows * W
                if jd == 0 and "xg0" in pref:
                    # first block: either the startup prefetch or the carry
                    # from the previous sc's boundary load
                    xg0, xg1 = pref.pop("xg0"), pref.pop("xg1")
                    rg0, rg1 = pref.pop("rg0"), pref.pop("rg1")
                elif jd == 16 and sc < 3:
                    # the 2-row tail IS the first 2 rows of the next sc's
                    # first block: load that block now and consume its head
                    for nm, src_d, pw_ in (("xg0", x_d, 128), ("xg1", x_d, 64),
                                           ("rg0", r_d, 128), ("rg1", r_d, 64)):
                        p0 = 0 if pw_ == 128 else 128
                        t = ing.tile([pw_, 1024], F32R, tag=nm, name=nm)
                        nc.sync.dma_start(
                            t[:], src_d[p0:p0 + pw_, nd:nd + 1024])
                        pref[nm] = t
                    xg0, xg1 = pref["xg0"], pref["xg1"]
                    rg0, rg1 = pref["rg0"], pref["rg1"]
                else:
                    xg0 = ing.tile([128, 1024], F32R, tag="xg0", name="xg0")
                    xg1 = ing.tile([64, 1024], F32R, tag="xg1", name="xg1")
                    rg0 = ing.tile([128, 1024], F32R, tag="rg0", name="rg0")
                    rg1 = ing.tile([64, 1024], F32R, tag="rg1", name="rg1")
                    nc.sync.dma_start(xg0[:, 0:dpix], x_d[0:128, nd:nd + dpix])
                    nc.sync.dma_start(xg1[:, 0:dpix], x_d[128:192, nd:nd + dpix])
                    nc.sync.dma_start(rg0[:, 0:dpix], r_d[0:128, nd:nd + dpix])
                    nc.sync.dma_start(rg1[:, 0:dpix], r_d[128:192, nd:nd + dpix])
                for js in range(0, drows, 4):
                    j0 = jd + js
                    nrows = min(4, drows - js)
                    npix = nrows * W
                    o0 = js * W
                    xrow = SC_OUT * sc + j0
                    for (co0, cow, zi, p0) in ((0, 128, 0, 0), (128, 64, 1, 0)):
                        ps = mps.tile([128, 512], F32, tag="main", name="cv_ps")
                        nc.tensor.matmul(ps[0:cow, 0:npix],
                                         lhsT=wq_sb0[:, co0:co0 + cow],
                                         rhs=xg0[:, o0:o0 + npix],
                                         start=True, stop=False)
                        nc.tensor.matmul(ps[0:cow, 0:npix],
                                         lhsT=wq_sb1[:, co0:co0 + cow],
                                         rhs=xg1[:, o0:o0 + npix],
                                         start=False, stop=True)
                        src = ps[0:cow, 0:npix].rearrange("p (h w) -> p h w", w=W)
                        copy_any(zdst(zsc[zi], j0, nrows, p0, cow), src)
                    kv_tiles = ((0, 64, ("sc", 1, 64)), (64, 128, ("sc", 2, 0)),
                                (192, 128, ("v", zv0, 0)), (320, 64, ("v", zv1, 0)))
                    for (co0, cow, dst) in kv_tiles:
                        ps = mps.tile([128, 512], F32, tag="main", name="cv_ps")
                        nc.tensor.matmul(ps[0:cow, 0:npix],
                                         lhsT=wkv_sb0[:, co0:co0 + cow],
                                         rhs=rg0[:, o0:o0 + npix],
                                         start=True, stop=False)
                        nc.tensor.matmul(ps[0:cow, 0:npix],
                                         lhsT=wkv_sb1[:, co0:co0 + cow],
                                         rhs=rg1[:, o0:o0 + npix],
                                         start=False, stop=True)
                        src = ps[0:cow, 0:npix].rearrange("p (h w) -> p h w", w=W)
                        if dst[0] == "sc":
                            copy_any(zdst(zsc[dst[1]], j0, nrows, dst[2], cow), src)
                        else:
                            copy_any(zdst(dst[1], xrow, nrows, dst[2], cow), src)

            if sc == 0:
                for (tile_, dsrc) in late_loads:
                    nc.scalar.dma_start(tile_[:], dsrc)
            if sc == 2:
                for (tile_, dsrc) in tail_loads:
                    nc.scalar.dma_start(tile_[:], dsrc)
            # duplicate this sc's zv1 rows into partitions 64:128 shifted by
            # -4 rows (slot r holds z-row r+4) for the packed v1 matmuls
            r0d, r1d = (4, 18) if sc == 0 else (16 * sc + 2, 16 * sc + 18)
            nc.sync.dma_start(
                zv1[64:128, ZBASE + ZSTRIDE * (r0d - 4):ZBASE + ZSTRIDE * (r1d - 4)],
                zv1[0:64, ZBASE + ZSTRIDE * r0d:ZBASE + ZSTRIDE * r1d])
            # --- dwconv + transpose + Gram for output rows 16sc..16sc+16 ---
            for pcc in range(SC_OUT // PC_ROWS):
                h0l = pcc * PC_ROWS
                h0g = SC_OUT * sc + h0l
                grans = []
                pcg = sc * 4 + pcc
                for c in range(3):
                    g = gcm.tile([128, 512], F32R, tag=f"g{c}", name=f"gcm{c}")
                    if c == 2:
                        # DVE path: balances PE (the overall bottleneck)
                        gv = g[:].rearrange("p (h w) -> p h w", w=W)
                        for t, (dh, dw) in enumerate(TAPS9):
                            iv = ztap(zsc[c], h0l, PC_ROWS, dh, dw)
                            if t == 0:
                                nc.vector.tensor_scalar(
                                    out=gv, in0=iv, scalar1=qkw_sb[c][:, 0:1],
                                    scalar2=None, op0=AL.mult)
                            else:
                                nc.vector.scalar_tensor_tensor(
                                    out=gv, in0=iv, scalar=qkw_sb[c][:, t:t + 1],
                                    in1=gv, op0=AL.mult, op1=AL.add)
                    else:
                        ps = mps.tile([128, 512], F32, tag="main", name="dw_ps")
                        for t, (dh, dw) in enumerate(TAPS9):
                            nc.tensor.matmul(
                                ps[:, :].rearrange("p (h w) -> p h w", w=W),
                                lhsT=dqk_sb[c][:, t * 128:(t + 1) * 128],
                                rhs=ztap(zsc[c], h0l, PC_ROWS, dh, dw),
                                start=(t == 0), stop=(t == 8))
                        copy_any(g[:], ps[:])
                    grans.append(g)
                # v0 rows 0:48 ride along per-pcc: the first 6 chains on PE
                # (DVE is the bottleneck in sc0/sc1), the last 6 on DVE
                # during sc2/sc3 where DVE has slack. Rows 48:64 go to the
                # PE collective window.
                if pcg < 4:
                    emit_v0_pe(4 * pcg)
                elif 10 <= pcg < 14:
                    emit_v0_dve(16 + 4 * (pcg - 10), PC_ROWS)
                if TRUNC < 3:
                    continue
                # full 128x128 transposes only (sliced is_transpose crashes);
                # pair-grouping happens in the PSUM->SBUF copy via strided src
                qkts = []
                for r in range(PC_ROWS):
                    qkt_ps = tps.tile([128, 384], F32R, tag="tp", name="qkt_ps")
                    for c in range(3):
                        nc.tensor.transpose(
                            qkt_ps[:, 128 * c:128 * (c + 1)],
                            grans[c][:, 128 * r:128 * (r + 1)],
                            ident_r[:])
                    qkt = qktp.tile([128, 384], F32R, tag="qkt", name="qkt")
                    srcv = qkt_ps[:].rearrange(
                        "p (g hp c) -> p hp g c", g=2, hp=3)
                    copy_any(qkt[:, :].rearrange(
                        "p (hp g c) -> p hp g c", hp=3, g=2), srcv)
                    qkts.append(qkt)
                # full-width Gram (N=384 keeps f32r at 1 cyc/row); only the
                # diagonal 128-block per m-tile is accumulated
                for m in range(3):
                    # 256-wide moving window containing the pair block keeps
                    # f32r at 1 cyc/row (needs N >= 256); accumulate straight
                    # into the persistent PSUM gacc slice
                    w0 = 128 * m if m < 2 else 128
                    for r in range(PC_ROWS):
                        nc.tensor.matmul(
                            gacc[:, 512 * m:512 * m + 256],
                            lhsT=qkts[r][:, 128 * m:128 * (m + 1)],
                            rhs=qkts[r][:, w0:w0 + 256],
                            start=(pcg == 0 and r == 0),
                            stop=(pcg == 15 and r == PC_ROWS - 1))

        if TRUNC >= 4:
            # ---------------- phase 3: stats + AllReduce ----------------
            # stats layout (DMA/view-friendly, 33-stride rows):
            #   [0:4224)    128 rows x 33: q-rows 0:128 (heads 0-3): attn|qsq
            #   [4224:6336)  64 rows x 33: q-rows 128:192 (heads 4,5)
            #   [6336:6528) k-norms as (head, d)
            # AllReduce(add) over the core pair sums both spatial halves, so
            # the consumer needs no adds — just 3 loads with direct views.
            stats_in = dram.tile([6528], F32)
            stats_out = dram.tile([2, 6528], F32)
            nrm = small.tile([128, 3], F32)
            junk = small.tile([128, 128], F32)
            pstage = small.tile([32, 6 * CH], F32, name="pstage")
            for hp in range(3):
                goff = 512 * hp + (0 if hp < 2 else 128)
                gv = gacc[:, goff:goff + 128]
                nc.vector.tensor_tensor(out=junk[:], in0=gv,
                                        in1=ident_f32[:], op=AL.mult)
                nc.vector.tensor_reduce(out=nrm[:, hp:hp + 1], in_=junk[:],
                                        axis=mybir.AxisListType.X, op=AL.add)
                for j, (r0, c0) in enumerate(((0, 64), (32, 96))):
                    head = 2 * hp + j
                    nc.scalar.copy(pstage[:, CH * head:CH * (head + 1)],
                                   gv[r0:r0 + 32, c0:c0 + 32])
            # 3 producer DMAs: attn rows (all 6 heads, contiguous 33-stride),
            # qsq column (one strided view over all 3 pair-chunks), k-norms
            nc.sync.dma_start(
                stats_in[0:6336].rearrange("(h c j) -> c h j", h=6, j=33)[:, :, 0:32],
                pstage[:].rearrange("p (h d) -> p h d", d=CH))
            nc.sync.dma_start(
                stats_in[0:6336].rearrange("(c p j) -> p c j", c=3, j=33)[:, :, 32:33],
                nrm[0:64, :, None])
            nc.sync.dma_start(
                stats_in[6336:6528].rearrange("(hp p) -> p hp", hp=3),
                nrm[64:128, :])
            if TRUNC == 35:
                nc.sync.dma_start(stats_out[0], stats_in[:])
                nc.sync.dma_start(stats_out[1], stats_in[:])
            else:
                nc.gpsimd.collective_compute(
                    "AllGather", AL.bypass,
                    replica_groups=[[0, 1], [2, 3], [4, 5], [6, 7]],
                    ins=[stats_in[:].opt()], outs=[stats_out[:].opt()])
            # fill the collective window: packed v1 (8 matmul chains) and v0
            # rows 40:64 on PE (6 chains)
            for pb in range(8):
                emit_v1_packed(8 * pb)
                emit_v0_pe(32 + 4 * pb)

            if TRUNC >= 5 and TRUNC != 35:
                # ---------------- phase 4: softmax with top-k rank masks ----------------
                # one wide DMA per region pulls BOTH gathered halves; one add
                # each sums them in place
                sumA = small.tile([128, 66], F32, name="sumA")
                sumB = small.tile([64, 66], F32, name="sumB")
                ksql2 = small.tile([HEADS, 2 * CH], F32, name="ksql2")
                nc.sync.dma_start(
                    sumA[:].rearrange("p (g j) -> p g j", j=33),
                    stats_out[:, 0:4224].rearrange("g (p j) -> p g j", j=33))
                nc.sync.dma_start(
                    sumB[:].rearrange("p (g j) -> p g j", j=33),
                    stats_out[:, 4224:6336].rearrange("g (p j) -> p g j", j=33))
                nc.sync.dma_start(
                    ksql2[:].rearrange("h (g d) -> h g d", d=CH),
                    stats_out[:, 6336:6528].rearrange("g (h d) -> h g d", d=CH))
                nc.vector.tensor_tensor(out=sumA[:, 0:33], in0=sumA[:, 0:33],
                                        in1=sumA[:, 33:66], op=AL.add)
                nc.vector.tensor_tensor(out=sumB[:, 0:33], in0=sumB[:, 0:33],
                                        in1=sumB[:, 33:66], op=AL.add)
                ksql = ksql2[:, 0:CH]
                nc.vector.tensor_tensor(out=ksql, in0=ksql,
                                        in1=ksql2[:, CH:2 * CH], op=AL.add)
                attn = [sumA[:, 0:CH], sumB[:, 0:CH]]
                qsq = [sumA[:, CH:CH + 1], sumB[:, CH:CH + 1]]
                # replicate ksq across rows of each head via tiny matmul
                ksqr = [small.tile([128, CH], F32, name="ksqr0"),
                        small.tile([64, CH], F32, name="ksqr1")]
                for i, (erep, pw) in enumerate(((erep0, 128), (erep1, 64))):
                    ps = mps.tile([128, 512], F32, tag="main", name="krep_ps")
                    nc.tensor.matmul(ps[0:pw, 0:CH], lhsT=erep[:], rhs=ksql,
                                     start=True, stop=True)
                    nc.vector.tensor_copy(ksqr[i][:], ps[0:pw, 0:CH])

                abd0 = small.tile([128, 192], BF16)
                abd1 = small.tile([64, 192], BF16)
                nc.vector.memset(abd0[:], 0.0)
                nc.vector.memset(abd1[:], 0.0)

                # Two chains: ti=0 big ops on DVE, ti=1 big ops on gpsimd/Pool
                # (parallel); tiny scalar prep + reduces stay on DVE; ACT ops
                # grouped by function so the table loads (sqrt, exp) happen
                # once each.
                s_cs, iks, exs, rks, cmps = [], [], [], [], []
                for ti, pw in ((0, 128), (1, 64)):
                    tempt = temp0 if ti == 0 else temp1
                    s_c = small.tile([128, 1], F32, tag=f"s{ti}", name="s_c")
                    ik = small.tile([128, CH], F32, tag=f"ik{ti}", name="ik")
                    nc.scalar.sqrt(s_c[0:pw, :], qsq[ti])
                    nc.scalar.sqrt(ik[0:pw, :], ksqr[ti][:])
                    nc.vector.tensor_scalar(out=s_c[0:pw, :], in0=s_c[0:pw, :],
                                            scalar1=1e-12, scalar2=None, op0=AL.max)
                    nc.vector.reciprocal(s_c[0:pw, :], s_c[0:pw, :])
                    nc.vector.tensor_mul(s_c[0:pw, :], s_c[0:pw, :], tempt[:])
                    nc.vector.tensor_scalar(out=ik[0:pw, :], in0=ik[0:pw, :],
                                            scalar1=1e-12, scalar2=None, op0=AL.max)
                    nc.vector.reciprocal(ik[0:pw, :], ik[0:pw, :])
                    s_cs.append(s_c)
                    iks.append(ik)
                for ti, pw in ((0, 128), (1, 64)):
                    ve = nc.vector
                    at = attn[ti]
                    # attn = P * (temp/|q|) * (1/|k|); Pool ucode has no
                    # TensorScalarPtr, so ti=1 uses two tensor_tensor ops
                    nc.vector.scalar_tensor_tensor(
                        out=at, in0=at, scalar=s_cs[ti][0:pw, 0:1],
                        in1=iks[ti][0:pw, :], op0=AL.mult, op1=AL.mult)
                    cmp = small.tile([128, CH * CH], BF16, tag=f"cmp{ti}", name="cmp")
                    c3 = cmp[0:pw, :].rearrange("p (j k) -> p j k", k=CH)
                    ve.tensor_tensor(
                        out=c3, in0=at[:, None, :].broadcast_to([pw, CH, CH]),
                        in1=at[:, :, None].broadcast_to([pw, CH, CH]), op=AL.is_ge)
                    cmps.append(c3)
                for ti, pw in ((0, 128), (1, 64)):
                    rk = small.tile([128, CH], F32, tag=f"rk{ti}", name="rk")
                    nc.vector.tensor_reduce(out=rk[0:pw, :], in_=cmps[ti],
                                            axis=mybir.AxisListType.X, op=AL.add)
                    nmax = small.tile([128, 1], F32, tag=f"nm{ti}", name="nmax")
                    nc.vector.tensor_reduce(out=nmax[0:pw, :], in_=attn[ti],
                                            axis=mybir.AxisListType.X, op=AL.max,
                                            negate=True)
                    ex = small.tile([128, CH], F32, tag=f"ex{ti}", name="ex")
                    nc.scalar.activation(ex[0:pw, :], attn[ti], AF.Exp,
                                         bias=nmax[0:pw, 0:1], scale=1.0)
                    rks.append(rk)
                    exs.append(ex)
                for ti, pw in ((0, 128), (1, 64)):
                    ve = nc.vector
                    awt = aw0 if ti == 0 else aw1
                    ksrt = ksr0 if ti == 0 else ksr1
                    rk, ex = rks[ti], exs[ti]
                    # all 4 top-k branches batched: mk4[p, i, j] = [rk_j <= k_i]
                    mk4 = small.tile([128, 4 * CH], BF16, tag=f"mk{ti}", name="mk4")
                    m3 = mk4[0:pw, :].rearrange("p (i j) -> p i j", j=CH)
                    ve.tensor_tensor(
                        out=m3, in0=rk[0:pw, None, :].broadcast_to([pw, 4, CH]),
                        in1=ksrt[:, :, None].broadcast_to([pw, 4, CH]), op=AL.is_le)
                    junk4 = small.tile([128, 4 * CH], F32, tag=f"jk{ti}", name="junk4")
                    j3 = junk4[0:pw, :].rearrange("p (i j) -> p i j", j=CH)
                    ve.tensor_tensor(
                        out=j3, in0=m3,
                        in1=ex[0:pw, None, :].broadcast_to([pw, 4, CH]), op=AL.mult)
                    sden4 = small.tile([128, 4], F32, tag=f"sd{ti}", name="sden4")
                    nc.vector.tensor_reduce(out=sden4[0:pw, :], in_=j3,
                                            axis=mybir.AxisListType.X, op=AL.add)
                    coef4 = small.tile([128, 4], F32, tag=f"cf{ti}", name="coef4")
                    nc.vector.reciprocal(coef4[0:pw, :], sden4[0:pw, :])
                    nc.vector.tensor_mul(coef4[0:pw, :], coef4[0:pw, :],
                                         awt[:, 0:4])
                    # gt = sum_i coef4[:, i] * mk4[:, i, :] (stt is DVE-only)
                    gt = small.tile([128, CH], F32, tag=f"gt{ti}", name="gt")
                    for i in range(4):
                        if i == 0:
                            nc.vector.tensor_scalar(
                                out=gt[0:pw, :],
                                in0=mk4[0:pw, i * CH:(i + 1) * CH],
                                scalar1=coef4[0:pw, i:i + 1],
                                scalar2=None, op0=AL.mult)
                        else:
                            nc.vector.scalar_tensor_tensor(
                                out=gt[0:pw, :], in0=mk4[0:pw, i * CH:(i + 1) * CH],
                                scalar=coef4[0:pw, i:i + 1], in1=gt[0:pw, :],
                                op0=AL.mult, op1=AL.add)
                    # A blocks into block-diagonal abd (bf16)
                    abdt = abd0 if ti == 0 else abd1
                    nheads_t = 4 if ti == 0 else 2
                    for j in range(nheads_t):
                        head = j if ti == 0 else 4 + j
                        ve.tensor_tensor(
                            out=abdt[32 * j:32 * (j + 1), 32 * head:32 * (head + 1)],
                            in0=ex[32 * j:32 * (j + 1), :],
                            in1=gt[32 * j:32 * (j + 1), :], op=AL.mult)

                if TRUNC == 5:
                    nc.gpsimd.dma_start(out_d[0:128, 0:192], abd0[:])
                    nc.gpsimd.dma_start(out_d[128:192, 0:192], abd1[:])
                if TRUNC >= 9:
                    # ---------------- phase 5: M2^T = A_bd^T @ Wp^T, final = M2 @ v ----------------
                    # m2tb0: v0-part of M2^T; m2tb1: v1-part, duplicated into
                    # partitions 64:128 so odd-group v1p matmuls line up.
                    m2tb0 = small.tile([128, 192], BF16, name="m2tb0")
                    m2tb1 = small.tile([128, 192], BF16, name="m2tb1")
                    for dt_i, (d0, dw_) in enumerate(((0, 128), (128, 64))):
                        ps = mps.tile([128, 512], F32, tag="main", name="m2_ps")
                        nc.tensor.matmul(ps[0:dw_, 0:192], lhsT=abd0[:, d0:d0 + dw_],
                                         rhs=wp0[:], start=True, stop=False)
                        nc.tensor.matmul(ps[0:dw_, 0:192], lhsT=abd1[:, d0:d0 + dw_],
                                         rhs=wp1[:], start=False, stop=True)
                        if dt_i == 0:
                            nc.vector.tensor_copy(m2tb0[:], ps[0:128, 0:192])
                        else:
                            nc.vector.tensor_copy(m2tb1[0:64, :], ps[0:64, 0:192])
                            nc.vector.tensor_copy(m2tb1[64:128, :], ps[0:64, 0:192])

                    FCH = 512
                    for ct, (co0, cw) in enumerate(((0, 128), (128, 64))):
                        for fc in range(NPX // FCH):
                            n0 = fc * FCH
                            po = (fc % 2) * 64
            # rotate over 6 PSUM banks (main 4 + gp 2; gp pads to a full
                            # bank anyway) so copy-out latency doesn't
                            # throttle the matmuls
                            if fc % 2 == 1:
                                ps = tps.tile([128, 512], F32, tag="tp",
                                              name="fo_ps2")
                            else:
                                ps = mps.tile([128, 512], F32, tag="main",
                                              name="fo_ps")
                            nc.tensor.matmul(ps[0:cw, :], lhsT=m2tb0[:, co0:co0 + cw],
                                             rhs=v0[:, n0:n0 + FCH], start=True, stop=False)
                            nc.tensor.matmul(ps[0:cw, :],
                                             lhsT=m2tb1[po:po + 64, co0:co0 + cw],
                                             rhs=v1p[po:po + 64,
                                                     (fc // 2) * FCH:(fc // 2 + 1) * FCH],
                                             start=False, stop=True)
                            if fc % 2 == 0:
                                fo = ing.tile([128, 2 * FCH], BF16, tag="fo",
                                              name="fo_sb", bufs=3)
                            # alternate ACT/DVE: both are idle in the tail,
                            # and 32 serial ACT copies would gate the final
                            fdst = fo[0:cw, (fc % 2) * FCH:(fc % 2 + 1) * FCH]
                            if fc % 2 == 0:
                                nc.scalar.copy(fdst, ps[0:cw, :])
                            else:
                                nc.vector.tensor_copy(fdst, ps[0:cw, :])
                            if fc % 2 == 1:
                                nc.sync.dma_start(
                                    out_d[co0:co0 + cw, n0 - FCH:n0 + FCH],
                                    fo[0:cw, :])

    nc.finalize()
    return nc


def _prep_inputs(x, ref, w_qkv, w_dw, w_proj, temperature, attn_w):
    bf = ml_dtypes.bfloat16
    w_qkv = np.asarray(w_qkv, np.float32)[:, :, 0, 0]          # [576, 192]
    w_dw = np.asarray(w_dw, np.float32)[:, 0]                  # [576, 3, 3]
    w_proj = np.asarray(w_proj, np.float32)[:, :, 0, 0]        # [192, 192]
    temp = np.asarray(temperature, np.float32).reshape(HEADS)
    aw = np.asarray(attn_w, np.float32).reshape(4)

    wq_t = np.ascontiguousarray(w_qkv[:192].T)                 # [ci, co]
    wkv_t = np.ascontiguousarray(w_qkv[192:].T)                # [ci, 384]

    dwq, dwk, dwv = w_dw[:192], w_dw[192:384], w_dw[384:]
    chunks = [dwq[0:128], np.concatenate([dwq[128:192], dwk[0:64]]), dwk[64:192]]
    dqk = np.zeros((3, 128, 9 * 128), np.float32)
    for c, blk in enumerate(chunks):
        for t, (dh, dw_) in enumerate(TAPS9):
            np.fill_diagonal(dqk[c, :, t * 128:(t + 1) * 128], blk[:, dh + 1, dw_ + 1])
    qkw = np.zeros((3, 128, 9), np.float32)
    for c, blk in enumerate(chunks):
        for t, (dh, dw_) in enumerate(TAPS9):
            qkw[c, :, t] = blk[:, dh + 1, dw_ + 1]
    vw = np.zeros((CDIM, 9), np.float32)
    for t, (dh, dw_) in enumerate(TAPS9):
        vw[:, t] = dwv[:, dh + 1, dw_ + 1]
    dv1 = np.zeros((128, 9 * 128), np.float32)
    for t, (dh, dw_) in enumerate(TAPS9):
        np.fill_diagonal(dv1[0:64, t * 128:t * 128 + 64],
                         dwv[128:192, dh + 1, dw_ + 1])
        np.fill_diagonal(dv1[64:128, t * 128 + 64:(t + 1) * 128],
                         dwv[128:192, dh + 1, dw_ + 1])
    dv0 = np.zeros((128, 9 * 128), np.float32)
    for t, (dh, dw_) in enumerate(TAPS9):
        np.fill_diagonal(dv0[:, t * 128:(t + 1) * 128], dwv[0:128, dh + 1, dw_ + 1])

    wp_t = np.ascontiguousarray(w_proj.T)                      # [c, co]
    temp_rep = np.repeat(temp, CH).reshape(CDIM, 1)
    aw_rep = np.tile(aw[None, :], (CDIM, 1))
    ks_rep = np.tile(np.asarray(KS_LIST, np.float32)[None, :], (CDIM, 1))
    ident = np.eye(128, dtype=np.float32)
    e0 = np.zeros((HEADS, 128), np.float32)
    e1 = np.zeros((HEADS, 64), np.float32)
    for h in range(4):
        e0[h, 32 * h:32 * (h + 1)] = 1.0
    for h in range(2):
        e1[h + 4, 32 * h:32 * (h + 1)] = 1.0

    xp = np.zeros((B, CDIM, H + 2, W), np.float32)
    xp[:, :, 1:H + 1] = np.asarray(x, np.float32)
    rp = np.zeros((B, CDIM, H + 2, W), np.float32)
    rp[:, :, 1:H + 1] = np.asarray(ref, np.float32)

    common = {
        "wq_t": wq_t, "wkv_t": wkv_t,
        "dqk": dqk, "vw": vw, "qkw": qkw, "dv1": dv1.astype(bf), "dv0": dv0.astype(bf),
        "wp_t": wp_t.astype(bf), "temp_rep": temp_rep, "aw_rep": aw_rep,
        "ks_rep": ks_rep,
        "ident_f32": ident,
        "e_rep0": e0, "e_rep1": e1,
    }
    in_maps = []
    for core in range(8):
        b, s = core // 2, core % 2
        m = dict(common)
        m["x_sh"] = np.ascontiguousarray(
            xp[b, :, 64 * s:64 * s + ROWS].reshape(CDIM, NIN))
        m["ref_sh"] = np.ascontiguousarray(
            rp[b, :, 64 * s:64 * s + ROWS].reshape(CDIM, NIN))
        in_maps.append(m)
    return in_maps


def _run(inputs, trace=False):
    if "nc" not in _CACHE:
        _CACHE["nc"] = _build()
    nc = _CACHE["nc"]
    in_maps = _prep_inputs(**inputs)
    res = run_bass_kernel_spmd(nc, in_maps, core_ids=list(range(8)), trace=trace)
    out = np.zeros((B, CDIM, H, W), np.float32)
    for core in range(8):
        b, s = core // 2, core % 2
        out[b, :, 64 * s:64 * (s + 1)] = np.asarray(
            res.results[core]["out"], np.float32).reshape(CDIM, HB, W)
    return out, res


def kernel(**inputs):
    out, _ = _run(inputs, trace=False)
    return out



# revision 143
# speedup vs baseline: 1.0200x; 1.0174x over previous
"""Trainium2 Bass kernel for sparse channel-attention (XCA-style) module.

Reference computation (b=4, c=192, h=w=128, heads=6, C=32):
  qkv  = dwconv3x3(conv1x1(x, w_qkv), w_dw); ref_qkv likewise (shared weights)
  q = qkv[:, :c] (from x), k = ref_qkv[:, c:2c], v = ref_qkv[:, 2c:]
  q,k L2-normalized along tokens; attn = (q @ k^T) * temperature  [b,6,32,32]
  out = sum_i attn_w[i] * softmax(topk-threshold(attn, k_i)) @ v;  proj conv1x1.

Sharding: 8 cores = (batch 0..3) x (spatial half 0..1, 64 rows + halo).
Cross-core traffic: one 26KB AllReduce per core pair (q/k norms + q@k^T).

Device algorithm per core:
  - conv1x1 via float32r matmuls (1024-px double-buffered input granules;
    first block prefetched ahead of the weight loads)
  - dwconv3x3 q,k: chunks 0,1 on TensorE as 9 PSUM-accumulated
    diag-matmuls, chunk 2 on VectorE stt chains (engine balance)
  - dwconv3x3 v: v1 (64ch) packed two 4-row granules per K=128 block-diag
    matmul via a partition-duplicated, 4-row-shifted zv1 copy; v0 split
    PE-early / DVE-late / PE-collective-window to level the engines
  - PE transposes q,k to token-major; Gram per head-pair accumulates
    directly in PSUM across all 16 chunks (bank-aligned slices — two
    accumulation regions must never share a PSUM bank)
  - AllGather(pair) of stats in a 33-stride row layout (attn|qsq per row)
    so the consumer needs 3 DMAs + 3 adds; softmax with top-k via rank
    counting, all 4 branches batched into single wide DVE ops
  - final = (w_proj @ A_blockdiag) @ v with fo staged bf16 and out DMAd
    bf16 (host converts back to f32); ACT sqrt table preloaded at start
    so the tail pays only the exp-table load
"""

from contextlib import ExitStack

import numpy as np
import ml_dtypes

import concourse.bass as bass
import concourse.mybir as mybir
import concourse.tile as tile
from concourse import bacc
from concourse.bass_utils import run_bass_kernel_spmd

F32 = mybir.dt.float32
F32R = mybir.dt.float32r
BF16 = mybir.dt.bfloat16
AL = mybir.AluOpType
AF = mybir.ActivationFunctionType

B, CDIM, H, W = 4, 192, 128, 128
HEADS, CH = 6, 32
HB = 64                      # rows per core (half image)
ROWS = HB + 2                # halo rows in z buffer (66)
ZSTRIDE = 130                # padded row stride in z (128 + 2 zero pad cols)
ZBASE = 2                    # leading guard elements in z tiles
ZLEN = ZBASE + ROWS * ZSTRIDE + 2   # 8584
NPX = HB * W                 # output pixels per core (8192)
NIN = ROWS * W               # conv input pixels per core (8448)
KS_LIST = [16, 21, 24, 25]   # top-k values for C=32
# tap order: dw=0 taps first (even parity for DVE 2x mode)
TAPS9 = [(-1, 0), (0, 0), (1, 0), (-1, -1), (-1, 1), (0, -1), (0, 1), (1, -1), (1, 1)]

_CACHE = {}
import os
TRUNC = int(os.environ.get("KTRUNC", "9"))


def _build():
    nc = bacc.Bacc("TRN2", num_devices=8, num_swdge_queues=4)

    # ---------------- kernel I/O ----------------
    x_d = nc.dram_tensor("x_sh", [CDIM, NIN], F32R, kind="ExternalInput")
    r_d = nc.dram_tensor("ref_sh", [CDIM, NIN], F32R, kind="ExternalInput")
    wq_d = nc.dram_tensor("wq_t", [CDIM, 192], F32R, kind="ExternalInput")
    wkv_d = nc.dram_tensor("wkv_t", [CDIM, 384], F32R, kind="ExternalInput")
    dqk_d = nc.dram_tensor("dqk", [3, 128, 9 * 128], F32R, kind="ExternalInput")
    vw_d = nc.dram_tensor("vw", [CDIM, 9], F32, kind="ExternalInput")
    dv1_d = nc.dram_tensor("dv1", [128, 9 * 128], BF16, kind="ExternalInput")
    qkw_d = nc.dram_tensor("qkw", [3, 128, 9], F32, kind="ExternalInput")
    dv0_d = nc.dram_tensor("dv0", [128, 9 * 128], BF16, kind="ExternalInput")
    wp_d = nc.dram_tensor("wp_t", [CDIM, 192], BF16, kind="ExternalInput")
    temp_d = nc.dram_tensor("temp_rep", [CDIM, 1], F32, kind="ExternalInput")
    aw_d = nc.dram_tensor("aw_rep", [CDIM, 4], F32, kind="ExternalInput")
    ksr_d = nc.dram_tensor("ks_rep", [CDIM, 4], F32, kind="ExternalInput")
    idf_d = nc.dram_tensor("ident_f32", [128, 128], F32, kind="ExternalInput")
    e0_d = nc.dram_tensor("e_rep0", [HEADS, 128], F32, kind="ExternalInput")
    e1_d = nc.dram_tensor("e_rep1", [HEADS, 64], F32, kind="ExternalInput")
    out_d = nc.dram_tensor("out", [CDIM, NPX], BF16, kind="ExternalOutput")

    with tile.TileContext(nc) as tc, ExitStack() as ctx:
        consts = ctx.enter_context(tc.tile_pool(name="consts", bufs=1))
        zpool = ctx.enter_context(tc.tile_pool(name="zpool", bufs=1))
        zscp = ctx.enter_context(tc.tile_pool(name="zscp", bufs=2))
        ing = ctx.enter_context(tc.tile_pool(name="ing", bufs=2))    # input granules
        gcm = ctx.enter_context(tc.tile_pool(name="gcm", bufs=2))    # qk chan-major granules
        qktp = ctx.enter_context(tc.tile_pool(name="qktp", bufs=6))  # token-major qk tiles
        small = ctx.enter_context(tc.tile_pool(name="small", bufs=1))
        mps = ctx.enter_context(tc.tile_pool(name="mps", bufs=3, space="PSUM"))
        tps = ctx.enter_context(tc.tile_pool(name="tps", bufs=2, space="PSUM"))
        gaccp = ctx.enter_context(tc.tile_pool(name="gaccp", bufs=1, space="PSUM"))
        dram = ctx.enter_context(tc.tile_pool(name="dram", bufs=1, space="DRAM"))

        # ---------------- constant loads ----------------
        # prefetch the first 8-row input block BEFORE the weight loads so
        # the HWDGE delivers it first and the PE can start ASAP
        pref = {}
        for nm, dsrc, pw_ in (("xg0", x_d[0:128, 0:1024], 128),
                              ("xg1", x_d[128:192, 0:1024], 64),
                              ("rg0", r_d[0:128, 0:1024], 128),
                              ("rg1", r_d[128:192, 0:1024], 64)):
            t = ing.tile([pw_, 1024], F32R, tag=nm, name=nm)
            nc.sync.dma_start(t[:], dsrc)
            pref[nm] = t
        wq_sb0 = consts.tile([128, 192], F32R)
        wq_sb1 = consts.tile([64, 192], F32R)
        wkv_sb0 = consts.tile([128, 384], F32R)
        wkv_sb1 = consts.tile([64, 384], F32R)
        # weight loads ride the ACT-triggered queue so the input-granule
        # streaming DMAs (SP queue) reach the HWDGE first
        nc.scalar.dma_start(wq_sb0[:], wq_d[0:128, :])
        nc.scalar.dma_start(wq_sb1[:], wq_d[128:192, :])
        nc.scalar.dma_start(wkv_sb0[:], wkv_d[0:128, :])
        nc.scalar.dma_start(wkv_sb1[:], wkv_d[128:192, :])
        # group A: needed from the first pcc (dwconv + transposes)
        late_loads = []
        # group B: needed only from the collective window onward
        tail_loads = []
        dqk_sb = []
        for c in range(3):
            t = consts.tile([128, 9 * 128], F32R, name=f"dqk_sb{c}")
            late_loads.append((t, dqk_d[c]))
            dqk_sb.append(t)
        vw0 = consts.tile([128, 9], F32)
        vw1 = consts.tile([64, 9], F32)
        dv1_sb = consts.tile([128, 9 * 128], BF16)
        tail_loads.append((dv1_sb, dv1_d[:]))
        dv0_sb = consts.tile([128, 9 * 128], BF16)
        late_loads.append((dv0_sb, dv0_d[:]))
        qkw_sb = []
        for c in range(3):
            t = consts.tile([128, 9], F32, name=f"qkw_sb{c}")
            late_loads.append((t, qkw_d[c]))
            qkw_sb.append(t)
        late_loads.append((vw0, vw_d[0:128, :]))
        late_loads.append((vw1, vw_d[128:192, :]))
        wp0 = consts.tile([128, 192], BF16)
        wp1 = consts.tile([64, 192], BF16)
        tail_loads.append((wp0, wp_d[0:128, :]))
        tail_loads.append((wp1, wp_d[128:192, :]))
        temp0 = consts.tile([128, 1], F32)
        temp1 = consts.tile([64, 1], F32)
        tail_loads.append((temp0, temp_d[0:128, :]))
        tail_loads.append((temp1, temp_d[128:192, :]))
        aw0 = consts.tile([128, 4], F32)
        aw1 = consts.tile([64, 4], F32)
        tail_loads.append((aw0, aw_d[0:128, :]))
        tail_loads.append((aw1, aw_d[128:192, :]))
        ksr0 = consts.tile([128, 4], F32)
        ksr1 = consts.tile([64, 4], F32)
        tail_loads.append((ksr0, ksr_d[0:128, :]))
        tail_loads.append((ksr1, ksr_d[128:192, :]))
        ident_f32 = consts.tile([128, 128], F32)
        ident_r = consts.tile([128, 128], F32R)
        tail_loads.append((ident_f32, idf_d[:]))
        late_loads.append((ident_r, idf_d[:].bitcast(F32R)))
        erep0 = consts.tile([HEADS, 128], F32)
        erep1 = consts.tile([HEADS, 64], F32)
        tail_loads.append((erep0, e0_d[:]))
        tail_loads.append((erep1, e1_d[:]))

        # ---------------- z buffers ----------------
        # q,k conv outputs (z) kept in f32 (bf16 z flips top-k ranks and blows
        # the error budget), held as rolling 16-row super-chunks to fit SBUF.
        # v z-buffer stays full-size bf16 (v precision barely matters).
        SC_OUT = 16
        SC_IN = SC_OUT + 2
        ZSCLEN = ZBASE + SC_IN * ZSTRIDE + 2
        # zv1 is [128, ...]: partitions 0:64 hold v-channels 128:192 for z-row
        # r at slot r; partitions 64:128 hold the SAME channels for z-row r+4
        # at slot r (a DMA-duplicated, 4-row-shifted copy). This lets the v1
        # dwconv run as full K=128/M=128 block-diag matmuls covering two
        # 4-row granules at once.
        zv0 = zpool.tile([128, ZLEN], BF16)
        zv1 = zpool.tile([128, ZLEN], BF16)
        v0 = zpool.tile([128, NPX], BF16)
        # v1p: packed v1 output [128, NPX/2]: partitions 0:64 = 4-row groups
        # 0,2,4,..., partitions 64:128 = groups 1,3,5,...
        v1p = zpool.tile([128, NPX // 2], BF16)
        for zt in (zv0, zv1):
            nc.gpsimd.memset(zt[:, 0:ZBASE], 0.0)
            pad = zt[:, ZBASE:ZBASE + ROWS * ZSTRIDE].rearrange(
                "p (h w) -> p h w", w=ZSTRIDE)[:, :, 128:130]
            nc.gpsimd.memset(pad, 0.0)

        # Touch Sqrt once so ACT's initial function table is
        # "sqrt_and_friends" (which also holds Copy) — the tail's sqrt then
        # needs no table reload in the post-collective critical path.
        warm = small.tile([1, 2], F32, name="warm")
        nc.vector.memset(warm[:], 1.0)
        nc.scalar.sqrt(warm[:, 0:1], warm[:, 1:2])

        ncopy = [0]

        def copy_any(dst, src):
            # spread copy load: ACT takes 3 of 4 (DVE carries the c2 + v0
            # dwconv chains during the main phase)
            use_dve = (ncopy[0] % 4 == 0)
            ncopy[0] += 1
            if use_dve:
                nc.vector.tensor_copy(dst, src)
            else:
                nc.scalar.copy(dst, src)

        def zdst(zt, j0, nrows, p0, pw):
            # strided view of z rows j0..j0+nrows (cols 0..127)
            v = zt[p0:p0 + pw, ZBASE + ZSTRIDE * j0: ZBASE + ZSTRIDE * (j0 + nrows)]
            return v.rearrange("p (h w) -> p h w", w=ZSTRIDE)[:, :, 0:128]

        def ztap(zt, h0, nrows, dh, dw):
            # read view for output rows h0..h0+nrows, tap (dh, dw)
            start = ZBASE + ZSTRIDE * (h0 + 1 + dh) + dw
            v = zt[:, start:start + ZSTRIDE * nrows]
            return v.rearrange("p (h w) -> p h w", w=ZSTRIDE)[:, :, 0:128]

        # G accumulates directly in PSUM across all 16 pccs (64-matmul
        # accumulation chains). Each head-pair chunk's 256-wide slice gets
        # its OWN bank (512-stride) — two concurrent accumulation regions
        # sharing a bank corrupt each other.
        gacc = gaccp.tile([128, 1536], F32, name="gacc")
        PC_ROWS = 4

        def emit_v1_packed(h0g):
            # two 4-row granules (h0g, h0g+4) in one K=128 block-diag matmul
            # chain; partitions 64:128 of zv1 hold the 4-row-shifted dup.
            ps = tps.tile([128, 512], F32, tag="tp", name="v1_ps")
            for t, (dh, dw) in enumerate(TAPS9):
                nc.tensor.matmul(
                    ps[:, :].rearrange("p (h w) -> p h w", w=W),
                    lhsT=dv1_sb[:, t * 128:(t + 1) * 128],
                    rhs=ztap(zv1, h0g, PC_ROWS, dh, dw),
                    start=(t == 0), stop=(t == 8))
            g = h0g // 8
            copy_any(v1p[:, g * 512:(g + 1) * 512], ps[:, :])

        def emit_v0_dve(h0, nrows):
            # v channels 0:128 on DVE (16-row chunks amortize the op init;
            # gpsimd ucode has no TensorScalarPtr so Pool can't take these)
            outv = v0[:, h0 * W:(h0 + nrows) * W].rearrange(
                "p (h w) -> p h w", w=W)
            for t, (dh, dw) in enumerate(TAPS9):
                iv = ztap(zv0, h0, nrows, dh, dw)
                if t == 0:
                    nc.vector.tensor_scalar(
                        out=outv, in0=iv, scalar1=vw0[:, 0:1],
                        scalar2=None, op0=AL.mult)
                else:
                    nc.vector.scalar_tensor_tensor(
                        out=outv, in0=iv, scalar=vw0[:, t:t + 1],
                        in1=outv, op0=AL.mult, op1=AL.add)

        def emit_v0_pe(h0g):
            # 4-row granule on PE diag-matmuls (fills the collective window)
            ps = mps.tile([128, 512], F32, tag="main", name="v0_ps")
            for t, (dh, dw) in enumerate(TAPS9):
                nc.tensor.matmul(
                    ps[:, :].rearrange("p (h w) -> p h w", w=W),
                    lhsT=dv0_sb[:, t * 128:(t + 1) * 128],
                    rhs=ztap(zv0, h0g, PC_ROWS, dh, dw),
                    start=(t == 0), stop=(t == 8))
            copy_any(v0[:, h0g * W:(h0g + PC_ROWS) * W], ps[:, :])

        for sc in range(4):
            # --- conv1x1 (f32r) for this super-chunk: 18 input rows ---
            zsc = []
            for c in range(3):
                t_ = zscp.tile([128, ZSCLEN], F32R, tag=f"zsc{c}", name=f"zsc{c}")
                nc.vector.memset(t_[:, 0:ZBASE].bitcast(F32), 0.0)
                padv = t_[:, ZBASE:ZBASE + SC_IN * ZSTRIDE].rearrange(
                    "p (h w) -> p h w", w=ZSTRIDE)[:, :, 128:130].bitcast(F32)
                nc.vector.memset(padv, 0.0)
                zsc.append(t_)
            for (jd, drows) in ((0, 8), (8, 8), (16, 2)):
                nd = (SC_OUT * sc + jd) * W
                dpix = drows * W
                if jd == 0 and "xg0" in pref:
                    # first block: either the startup prefetch or the carry
                    # from the previous sc's boundary load
                    xg0, xg1 = pref.pop("xg0"), pref.pop("xg1")
                    rg0, rg1 = pref.pop("rg0"), pref.pop("rg1")
                elif jd == 16 and sc < 3:
                    # the 2-row tail IS the first 2 rows of the next sc's
                    # first block: load that block now and consume its head
                    for nm, src_d, pw_ in (("xg0", x_d, 128), ("xg1", x_d, 64),
                                           ("rg0", r_d, 128), ("rg1", r_d, 64)):
                        p0 = 0 if pw_ == 128 else 128
                        t = ing.tile([pw_, 1024], F32R, tag=nm, name=nm)
                        nc.sync.dma_start(
                            t[:], src_d[p0:p0 + pw_, nd:nd + 1024])
                        pref[nm] = t
                    xg0, xg1 = pref["xg0"], pref["xg1"]
                    rg0, rg1 = pref["rg0"], pref["rg1"]
                else:
                    xg0 = ing.tile([128, 1024], F32R, tag="xg0", name="xg0")
                    xg1 = ing.tile([64, 1024], F32R, tag="xg1", name="xg1")
                    rg0 = ing.tile([128, 1024], F32R, tag="rg0", name="rg0")
                    rg1 = ing.tile([64, 1024], F32R, tag="rg1", name="rg1")
                    nc.sync.dma_start(xg0[:, 0:dpix], x_d[0:128, nd:nd + dpix])
                    nc.sync.dma_start(xg1[:, 0:dpix], x_d[128:192, nd:nd + dpix])
                    nc.sync.dma_start(rg0[:, 0:dpix], r_d[0:128, nd:nd + dpix])
                    nc.sync.dma_start(rg1[:, 0:dpix], r_d[128:192, nd:nd + dpix])
                for js in range(0, drows, 4):
                    j0 = jd + js
                    nrows = min(4, drows - js)
                    npix = nrows * W
                    o0 = js * W
                    xrow = SC_OUT * sc + j0
                    for (co0, cow, zi, p0) in ((0, 128, 0, 0), (128, 64, 1, 0)):
                        ps = mps.tile([128, 512], F32, tag="main", name="cv_ps")
                        nc.tensor.matmul(ps[0:cow, 0:npix],
                                         lhsT=wq_sb0[:, co0:co0 + cow],
                                         rhs=xg0[:, o0:o0 + npix],
                                         start=True, stop=False)
                        nc.tensor.matmul(ps[0:cow, 0:npix],
                                         lhsT=wq_sb1[:, co0:co0 + cow],
                                         rhs=xg1[:, o0:o0 + npix],
                                         start=False, stop=True)
                        src = ps[0:cow, 0:npix].rearrange("p (h w) -> p h w", w=W)
                        copy_any(zdst(zsc[zi], j0, nrows, p0, cow), src)
                    kv_tiles = ((0, 64, ("sc", 1, 64)), (64, 128, ("sc", 2, 0)),
                                (192, 128, ("v", zv0, 0)), (320, 64, ("v", zv1, 0)))
                    for kvi, (co0, cow, dst) in enumerate(kv_tiles):
                        if kvi >= 2:
                            ps = tps.tile([128, 512], F32, tag="tp",
                                          name="cv_ps2")
                        else:
                            ps = mps.tile([128, 512], F32, tag="main",
                                          name="cv_ps")
                        nc.tensor.matmul(ps[0:cow, 0:npix],
                                         lhsT=wkv_sb0[:, co0:co0 + cow],
                                         rhs=rg0[:, o0:o0 + npix],
                                         start=True, stop=False)
                        nc.tensor.matmul(ps[0:cow, 0:npix],
                                         lhsT=wkv_sb1[:, co0:co0 + cow],
                                         rhs=rg1[:, o0:o0 + npix],
                                         start=False, stop=True)
                        src = ps[0:cow, 0:npix].rearrange("p (h w) -> p h w", w=W)
                        if dst[0] == "sc":
                            copy_any(zdst(zsc[dst[1]], j0, nrows, dst[2], cow), src)
                        else:
                            copy_any(zdst(dst[1], xrow, nrows, dst[2], cow), src)

            if sc == 0:
                for (tile_, dsrc) in late_loads:
                    nc.scalar.dma_start(tile_[:], dsrc)
            if sc == 2:
                for (tile_, dsrc) in tail_loads:
                    nc.scalar.dma_start(tile_[:], dsrc)
            # duplicate this sc's zv1 rows into partitions 64:128 shifted by
            # -4 rows (slot r holds z-row r+4) for the packed v1 matmuls
            r0d, r1d = (4, 18) if sc == 0 else (16 * sc + 2, 16 * sc + 18)
            nc.sync.dma_start(
                zv1[64:128, ZBASE + ZSTRIDE * (r0d - 4):ZBASE + ZSTRIDE * (r1d - 4)],
                zv1[0:64, ZBASE + ZSTRIDE * r0d:ZBASE + ZSTRIDE * r1d])
            # --- dwconv + transpose + Gram for output rows 16sc..16sc+16 ---
            for pcc in range(SC_OUT // PC_ROWS):
                h0l = pcc * PC_ROWS
                h0g = SC_OUT * sc + h0l
                grans = []
                pcg = sc * 4 + pcc
                for c in range(3):
                    g = gcm.tile([128, 512], F32R, tag=f"g{c}", name=f"gcm{c}")
                    if c == 2:
                        # DVE path: balances PE (the overall bottleneck)
                        gv = g[:].rearrange("p (h w) -> p h w", w=W)
                        for t, (dh, dw) in enumerate(TAPS9):
                            iv = ztap(zsc[c], h0l, PC_ROWS, dh, dw)
                            if t == 0:
                                nc.vector.tensor_scalar(
                                    out=gv, in0=iv, scalar1=qkw_sb[c][:, 0:1],
                                    scalar2=None, op0=AL.mult)
                            else:
                                nc.vector.scalar_tensor_tensor(
                                    out=gv, in0=iv, scalar=qkw_sb[c][:, t:t + 1],
                                    in1=gv, op0=AL.mult, op1=AL.add)
                    else:
                        ps = mps.tile([128, 512], F32, tag="main", name="dw_ps")
                        for t, (dh, dw) in enumerate(TAPS9):
                            nc.tensor.matmul(
                                ps[:, :].rearrange("p (h w) -> p h w", w=W),
                                lhsT=dqk_sb[c][:, t * 128:(t + 1) * 128],
                                rhs=ztap(zsc[c], h0l, PC_ROWS, dh, dw),
                                start=(t == 0), stop=(t == 8))
                        copy_any(g[:], ps[:])
                    grans.append(g)
                # v0 rows 0:48 ride along per-pcc: the first 6 chains on PE
                # (DVE is the bottleneck in sc0/sc1), the last 6 on DVE
                # during sc2/sc3 where DVE has slack. Rows 48:64 go to the
                # PE collective window.
                if pcg < 4:
                    emit_v0_pe(4 * pcg)
                elif 10 <= pcg < 14:
                    emit_v0_dve(16 + 4 * (pcg - 10), PC_ROWS)
                if TRUNC < 3:
                    continue
                # full 128x128 transposes only (sliced is_transpose crashes);
                # pair-grouping happens in the PSUM->SBUF copy via strided src
                qkts = []
                for r in range(PC_ROWS):
                    qkt_ps = tps.tile([128, 384], F32R, tag="tp", name="qkt_ps")
                    for c in range(3):
                        nc.tensor.transpose(
                            qkt_ps[:, 128 * c:128 * (c + 1)],
                            grans[c][:, 128 * r:128 * (r + 1)],
                            ident_r[:])
                    qkt = qktp.tile([128, 384], F32R, tag="qkt", name="qkt")
                    srcv = qkt_ps[:].rearrange(
                        "p (g hp c) -> p hp g c", g=2, hp=3)
                    copy_any(qkt[:, :].rearrange(
                        "p (hp g c) -> p hp g c", hp=3, g=2), srcv)
                    qkts.append(qkt)
                # full-width Gram (N=384 keeps f32r at 1 cyc/row); only the
                # diagonal 128-block per m-tile is accumulated
                for m in range(3):
                    # 256-wide moving window containing the pair block keeps
                    # f32r at 1 cyc/row (needs N >= 256); accumulate straight
                    # into the persistent PSUM gacc slice
                    w0 = 128 * m if m < 2 else 128
                    for r in range(PC_ROWS):
                        nc.tensor.matmul(
                            gacc[:, 512 * m:512 * m + 256],
                            lhsT=qkts[r][:, 128 * m:128 * (m + 1)],
                            rhs=qkts[r][:, w0:w0 + 256],
                            start=(pcg == 0 and r == 0),
                            stop=(pcg == 15 and r == PC_ROWS - 1))

        if TRUNC >= 4:
            # ---------------- phase 3: stats + AllReduce ----------------
            # stats layout (DMA/view-friendly, 33-stride rows):
            #   [0:4224)    128 rows x 33: q-rows 0:128 (heads 0-3): attn|qsq
            #   [4224:6336)  64 rows x 33: q-rows 128:192 (heads 4,5)
            #   [6336:6528) k-norms as (head, d)
            # AllReduce(add) over the core pair sums both spatial halves, so
            # the consumer needs no adds — just 3 loads with direct views.
            stats_in = dram.tile([6528], F32)
            stats_out = dram.tile([2, 6528], F32)
            nrm = small.tile([128, 3], F32)
            junk = small.tile([128, 128], F32)
            pstage = small.tile([32, 6 * CH], F32, name="pstage")
            for hp in range(3):
                goff = 512 * hp + (0 if hp < 2 else 128)
                gv = gacc[:, goff:goff + 128]
                nc.vector.tensor_tensor(out=junk[:], in0=gv,
                                        in1=ident_f32[:], op=AL.mult)
                nc.vector.tensor_reduce(out=nrm[:, hp:hp + 1], in_=junk[:],
                                        axis=mybir.AxisListType.X, op=AL.add)
                for j, (r0, c0) in enumerate(((0, 64), (32, 96))):
                    head = 2 * hp + j
                    nc.scalar.copy(pstage[:, CH * head:CH * (head + 1)],
                                   gv[r0:r0 + 32, c0:c0 + 32])
            # 3 producer DMAs: attn rows (all 6 heads, contiguous 33-stride),
            # qsq column (one strided view over all 3 pair-chunks), k-norms
            nc.sync.dma_start(
                stats_in[0:6336].rearrange("(h c j) -> c h j", h=6, j=33)[:, :, 0:32],
                pstage[:].rearrange("p (h d) -> p h d", d=CH))
            nc.sync.dma_start(
                stats_in[0:6336].rearrange("(c p j) -> p c j", c=3, j=33)[:, :, 32:33],
                nrm[0:64, :, None])
            nc.sync.dma_start(
                stats_in[6336:6528].rearrange("(hp p) -> p hp", hp=3),
                nrm[64:128, :])
            if TRUNC == 35:
                nc.sync.dma_start(stats_out[0], stats_in[:])
                nc.sync.dma_start(stats_out[1], stats_in[:])
            else:
                nc.gpsimd.collective_compute(
                    "AllGather", AL.bypass,
                    replica_groups=[[0, 1], [2, 3], [4, 5], [6, 7]],
                    ins=[stats_in[:].opt()], outs=[stats_out[:].opt()])
            # fill the collective window: packed v1 (8 matmul chains) and v0
            # rows 40:64 on PE (6 chains)
            for pb in range(8):
                emit_v1_packed(8 * pb)
                emit_v0_pe(32 + 4 * pb)

            if TRUNC >= 5 and TRUNC != 35:
                # ---------------- phase 4: softmax with top-k rank masks ----------------
                # one wide DMA per region pulls BOTH gathered halves; one add
                # each sums them in place
                sumA = small.tile([128, 66], F32, name="sumA")
                sumB = small.tile([64, 66], F32, name="sumB")
                ksql2 = small.tile([HEADS, 2 * CH], F32, name="ksql2")
                nc.sync.dma_start(
                    sumA[:].rearrange("p (g j) -> p g j", j=33),
                    stats_out[:, 0:4224].rearrange("g (p j) -> p g j", j=33))
                nc.sync.dma_start(
                    sumB[:].rearrange("p (g j) -> p g j", j=33),
                    stats_out[:, 4224:6336].rearrange("g (p j) -> p g j", j=33))
                nc.sync.dma_start(
                    ksql2[:].rearrange("h (g d) -> h g d", d=CH),
                    stats_out[:, 6336:6528].rearrange("g (h d) -> h g d", d=CH))
                nc.vector.tensor_tensor(out=sumA[:, 0:33], in0=sumA[:, 0:33],
                                        in1=sumA[:, 33:66], op=AL.add)
                nc.vector.tensor_tensor(out=sumB[:, 0:33], in0=sumB[:, 0:33],
                                        in1=sumB[:, 33:66], op=AL.add)
                ksql = ksql2[:, 0:CH]
                nc.vector.tensor_tensor(out=ksql, in0=ksql,
                                        in1=ksql2[:, CH:2 * CH], op=AL.add)
                attn = [sumA[:, 0:CH], sumB[:, 0:CH]]
                qsq = [sumA[:, CH:CH + 1], sumB[:, CH:CH + 1]]
                # replicate ksq across rows of each head via tiny matmul
                ksqr = [small.tile([128, CH], F32, name="ksqr0"),
                        small.tile([64, CH], F32, name="ksqr1")]
                for i, (erep, pw) in enumerate(((erep0, 128), (erep1, 64))):
                    ps = mps.tile([128, 512], F32, tag="main", name="krep_ps")
                    nc.tensor.matmul(ps[0:pw, 0:CH], lhsT=erep[:], rhs=ksql,
                                     start=True, stop=True)
                    nc.vector.tensor_copy(ksqr[i][:], ps[0:pw, 0:CH])

                abd0 = small.tile([128, 192], BF16)
                abd1 = small.tile([64, 192], BF16)
                nc.vector.memset(abd0[:], 0.0)
                nc.vector.memset(abd1[:], 0.0)

                # Two chains: ti=0 big ops on DVE, ti=1 big ops on gpsimd/Pool
                # (parallel); tiny scalar prep + reduces stay on DVE; ACT ops
                # grouped by function so the table loads (sqrt, exp) happen
                # once each.
                s_cs, iks, exs, rks, cmps = [], [], [], [], []
                for ti, pw in ((0, 128), (1, 64)):
                    tempt = temp0 if ti == 0 else temp1
                    s_c = small.tile([128, 1], F32, tag=f"s{ti}", name="s_c")
                    ik = small.tile([128, CH], F32, tag=f"ik{ti}", name="ik")
                    nc.scalar.sqrt(s_c[0:pw, :], qsq[ti])
                    nc.scalar.sqrt(ik[0:pw, :], ksqr[ti][:])
                    nc.vector.tensor_scalar(out=s_c[0:pw, :], in0=s_c[0:pw, :],
                                            scalar1=1e-12, scalar2=None, op0=AL.max)
                    nc.vector.reciprocal(s_c[0:pw, :], s_c[0:pw, :])
                    nc.vector.tensor_mul(s_c[0:pw, :], s_c[0:pw, :], tempt[:])
                    nc.vector.tensor_scalar(out=ik[0:pw, :], in0=ik[0:pw, :],
                                            scalar1=1e-12, scalar2=None, op0=AL.max)
                    nc.vector.reciprocal(ik[0:pw, :], ik[0:pw, :])
                    s_cs.append(s_c)
                    iks.append(ik)
                for ti, pw in ((0, 128), (1, 64)):
                    ve = nc.vector
                    at = attn[ti]
                    # attn = P * (temp/|q|) * (1/|k|); Pool ucode has no
                    # TensorScalarPtr, so ti=1 uses two tensor_tensor ops
                    nc.vector.scalar_tensor_tensor(
                        out=at, in0=at, scalar=s_cs[ti][0:pw, 0:1],
                        in1=iks[ti][0:pw, :], op0=AL.mult, op1=AL.mult)
                    cmp = small.tile([128, CH * CH], BF16, tag=f"cmp{ti}", name="cmp")
                    c3 = cmp[0:pw, :].rearrange("p (j k) -> p j k", k=CH)
                    ve.tensor_tensor(
                        out=c3, in0=at[:, None, :].broadcast_to([pw, CH, CH]),
                        in1=at[:, :, None].broadcast_to([pw, CH, CH]), op=AL.is_ge)
                    cmps.append(c3)
                for ti, pw in ((0, 128), (1, 64)):
                    rk = small.tile([128, CH], F32, tag=f"rk{ti}", name="rk")
                    nc.vector.tensor_reduce(out=rk[0:pw, :], in_=cmps[ti],
                                            axis=mybir.AxisListType.X, op=AL.add)
                    nmax = small.tile([128, 1], F32, tag=f"nm{ti}", name="nmax")
                    nc.vector.tensor_reduce(out=nmax[0:pw, :], in_=attn[ti],
                                            axis=mybir.AxisListType.X, op=AL.max,
                                            negate=True)
                    ex = small.tile([128, CH], F32, tag=f"ex{ti}", name="ex")
                    nc.scalar.activation(ex[0:pw, :], attn[ti], AF.Exp,
                                         bias=nmax[0:pw, 0:1], scale=1.0)
                    rks.append(rk)
                    exs.append(ex)
                for ti, pw in ((0, 128), (1, 64)):
                    ve = nc.vector
                    awt = aw0 if ti == 0 else aw1
                    ksrt = ksr0 if ti == 0 else ksr1
                    rk, ex = rks[ti], exs[ti]
                    # all 4 top-k branches batched: mk4[p, i, j] = [rk_j <= k_i]
                    mk4 = small.tile([128, 4 * CH], BF16, tag=f"mk{ti}", name="mk4")
                    m3 = mk4[0:pw, :].rearrange("p (i j) -> p i j", j=CH)
                    ve.tensor_tensor(
                        out=m3, in0=rk[0:pw, None, :].broadcast_to([pw, 4, CH]),
                        in1=ksrt[:, :, None].broadcast_to([pw, 4, CH]), op=AL.is_le)
                    junk4 = small.tile([128, 4 * CH], F32, tag=f"jk{ti}", name="junk4")
                    j3 = junk4[0:pw, :].rearrange("p (i j) -> p i j", j=CH)
                    ve.tensor_tensor(
                        out=j3, in0=m3,
                        in1=ex[0:pw, None, :].broadcast_to([pw, 4, CH]), op=AL.mult)
                    sden4 = small.tile([128, 4], F32, tag=f"sd{ti}", name="sden4")
                    nc.vector.tensor_reduce(out=sden4[0:pw, :], in_=j3,
                                            axis=mybir.AxisListType.X, op=AL.add)
                    coef4 = small.tile([128, 4], F32, tag=f"cf{ti}", name="coef4")
                    nc.vector.reciprocal(coef4[0:pw, :], sden4[0:pw, :])
                    nc.vector.tensor_mul(coef4[0:pw, :], coef4[0:pw, :],
                                         awt[:, 0:4])
                    # gt = sum_i coef4[:, i] * mk4[:, i, :] (stt is DVE-only)
                    gt = small.tile([128, CH], F32, tag=f"gt{ti}", name="gt")
                    for i in range(4):
                        if i == 0:
                            nc.vector.tensor_scalar(
                                out=gt[0:pw, :],
                                in0=mk4[0:pw, i * CH:(i + 1) * CH],
                                scalar1=coef4[0:pw, i:i + 1],
                                scalar2=None, op0=AL.mult)
                        else:
                            nc.vector.scalar_tensor_tensor(
                                out=gt[0:pw, :], in0=mk4[0:pw, i * CH:(i + 1) * CH],
                                scalar=coef4[0:pw, i:i + 1], in1=gt[0:pw, :],
                                op0=AL.mult, op1=AL.add)
                    # A blocks into block-diagonal abd (bf16)
                    abdt = abd0 if ti == 0 else abd1
                    nheads_t = 4 if ti == 0 else 2
                    for j in range(nheads_t):
                        head = j if ti == 0 else 4 + j
                        ve.tensor_tensor(
                            out=abdt[32 * j:32 * (j + 1), 32 * head:32 * (head + 1)],
                            in0=ex[32 * j:32 * (j + 1), :],
                            in1=gt[32 * j:32 * (j + 1), :], op=AL.mult)

                if TRUNC == 5:
                    nc.gpsimd.dma_start(out_d[0:128, 0:192], abd0[:])
                    nc.gpsimd.dma_start(out_d[128:192, 0:192], abd1[:])
                if TRUNC >= 9:
                    # ---------------- phase 5: M2^T = A_bd^T @ Wp^T, final = M2 @ v ----------------
                    # m2tb0: v0-part of M2^T; m2tb1: v1-part, duplicated into
                    # partitions 64:128 so odd-group v1p matmuls line up.
                    m2tb0 = small.tile([128, 192], BF16, name="m2tb0")
                    m2tb1 = small.tile([128, 192], BF16, name="m2tb1")
                    for dt_i, (d0, dw_) in enumerate(((0, 128), (128, 64))):
                        ps = mps.tile([128, 512], F32, tag="main", name="m2_ps")
                        nc.tensor.matmul(ps[0:dw_, 0:192], lhsT=abd0[:, d0:d0 + dw_],
                                         rhs=wp0[:], start=True, stop=False)
                        nc.tensor.matmul(ps[0:dw_, 0:192], lhsT=abd1[:, d0:d0 + dw_],
                                         rhs=wp1[:], start=False, stop=True)
                        if dt_i == 0:
                            nc.vector.tensor_copy(m2tb0[:], ps[0:128, 0:192])
                        else:
                            nc.vector.tensor_copy(m2tb1[0:64, :], ps[0:64, 0:192])
                            nc.vector.tensor_copy(m2tb1[64:128, :], ps[0:64, 0:192])

                    FCH = 512
                    for ct, (co0, cw) in enumerate(((0, 128), (128, 64))):
                        for fc in range(NPX // FCH):
                            n0 = fc * FCH
                            po = (fc % 2) * 64
            # rotate over 6 PSUM banks (main 4 + gp 2; gp pads to a full
                            # bank anyway) so copy-out latency doesn't
                            # throttle the matmuls
                            if fc % 2 == 1:
                                ps = tps.tile([128, 512], F32, tag="tp",
                                              name="fo_ps2")
                            else:
                                ps = mps.tile([128, 512], F32, tag="main",
                                              name="fo_ps")
                            nc.tensor.matmul(ps[0:cw, :], lhsT=m2tb0[:, co0:co0 + cw],
                                             rhs=v0[:, n0:n0 + FCH], start=True, stop=False)
                            nc.tensor.matmul(ps[0:cw, :],
                                             lhsT=m2tb1[po:po + 64, co0:co0 + cw],
                                             rhs=v1p[po:po + 64,
                                                     (fc // 2) * FCH:(fc // 2 + 1) * FCH],
                                             start=False, stop=True)
                            if fc % 2 == 0:
                                fo = ing.tile([128, 2 * FCH], BF16, tag="fo",
                                              name="fo_sb", bufs=3)
                            # alternate ACT/DVE: both are idle in the tail,
                            # and 32 serial ACT copies would gate the final
                            fdst = fo[0:cw, (fc % 2) * FCH:(fc % 2 + 1) * FCH]
                            if fc % 2 == 0:
                                nc.scalar.copy(fdst, ps[0:cw, :])
                            else:
                                nc.vector.tensor_copy(fdst, ps[0:cw, :])
                            if fc % 2 == 1:
                                nc.sync.dma_start(
                                    out_d[co0:co0 + cw, n0 - FCH:n0 + FCH],
                                    fo[0:cw, :])

    nc.finalize()
    return nc


def _prep_inputs(x, ref, w_qkv, w_dw, w_proj, temperature, attn_w):
    bf = ml_dtypes.bfloat16
    w_qkv = np.asarray(w_qkv, np.float32)[:, :, 0, 0]          # [576, 192]
    w_dw = np.asarray(w_dw, np.float32)[:, 0]                  # [576, 3, 3]
    w_proj = np.asarray(w_proj, np.float32)[:, :, 0, 0]        # [192, 192]
    temp = np.asarray(temperature, np.float32).reshape(HEADS)
    aw = np.asarray(attn_w, np.float32).reshape(4)

    wq_t = np.ascontiguousarray(w_qkv[:192].T)                 # [ci, co]
    wkv_t = np.ascontiguousarray(w_qkv[192:].T)                # [ci, 384]

    dwq, dwk, dwv = w_dw[:192], w_dw[192:384], w_dw[384:]
    chunks = [dwq[0:128], np.concatenate([dwq[128:192], dwk[0:64]]), dwk[64:192]]
    dqk = np.zeros((3, 128, 9 * 128), np.float32)
    for c, blk in enumerate(chunks):
        for t, (dh, dw_) in enumerate(TAPS9):
            np.fill_diagonal(dqk[c, :, t * 128:(t + 1) * 128], blk[:, dh + 1, dw_ + 1])
    qkw = np.zeros((3, 128, 9), np.float32)
    for c, blk in enumerate(chunks):
        for t, (dh, dw_) in enumerate(TAPS9):
            qkw[c, :, t] = blk[:, dh + 1, dw_ + 1]
    vw = np.zeros((CDIM, 9), np.float32)
    for t, (dh, dw_) in enumerate(TAPS9):
        vw[:, t] = dwv[:, dh + 1, dw_ + 1]
    dv1 = np.zeros((128, 9 * 128), np.float32)
    for t, (dh, dw_) in enumerate(TAPS9):
        np.fill_diagonal(dv1[0:64, t * 128:t * 128 + 64],
                         dwv[128:192, dh + 1, dw_ + 1])
        np.fill_diagonal(dv1[64:128, t * 128 + 64:(t + 1) * 128],
                         dwv[128:192, dh + 1, dw_ + 1])
    dv0 = np.zeros((128, 9 * 128), np.float32)
    for t, (dh, dw_) in enumerate(TAPS9):
        np.fill_diagonal(dv0[:, t * 128:(t + 1) * 128], dwv[0:128, dh + 1, dw_ + 1])

    wp_t = np.ascontiguousarray(w_proj.T)                      # [c, co]
    temp_rep = np.repeat(temp, CH).reshape(CDIM, 1)
    aw_rep = np.tile(aw[None, :], (CDIM, 1))
    ks_rep = np.tile(np.asarray(KS_LIST, np.float32)[None, :], (CDIM, 1))
    ident = np.eye(128, dtype=np.float32)
    e0 = np.zeros((HEADS, 128), np.float32)
    e1 = np.zeros((HEADS, 64), np.float32)
    for h in range(4):
        e0[h, 32 * h:32 * (h + 1)] = 1.0
    for h in range(2):
        e1[h + 4, 32 * h:32 * (h + 1)] = 1.0

    xp = np.zeros((B, CDIM, H + 2, W), np.float32)
    xp[:, :, 1:H + 1] = np.asarray(x, np.float32)
    rp = np.zeros((B, CDIM, H + 2, W), np.float32)
    rp[:, :, 1:H + 1] = np.asarray(ref, np.float32)

    common = {
        "wq_t": wq_t, "wkv_t": wkv_t,
        "dqk": dqk, "vw": vw, "qkw": qkw, "dv1": dv1.astype(bf), "dv0": dv0.astype(bf),
        "wp_t": wp_t.astype(bf), "temp_rep": temp_rep, "aw_rep": aw_rep,
        "ks_rep": ks_rep,
        "ident_f32": ident,
        "e_rep0": e0, "e_rep1": e1,
    }
    in_maps = []
    for core in range(8):
        b, s = core // 2, core % 2
        m = dict(common)
        m["x_sh"] = np.ascontiguousarray(
            xp[b, :, 64 * s:64 * s + ROWS].reshape(CDIM, NIN))
        m["ref_sh"] = np.ascontiguousarray(
            rp[b, :, 64 * s:64 * s + ROWS].reshape(CDIM, NIN))
        in_maps.append(m)
    return in_maps


def _run(inputs, trace=False):
    if "nc" not in _CACHE:
        _CACHE["nc"] = _build()
    nc = _CACHE["nc"]
    in_maps = _prep_inputs(**inputs)
    res = run_bass_kernel_spmd(nc, in_maps, core_ids=list(range(8)), trace=trace)
    out = np.zeros((B, CDIM, H, W), np.float32)
    for core in range(8):
        b, s = core // 2, core % 2
        out[b, :, 64 * s:64 * (s + 1)] = np.asarray(
            res.results[core]["out"], np.float32).reshape(CDIM, HB, W)
    return out, res


def kernel(**inputs):
    out, _ = _run(inputs, trace=False)
    return out



# revision 158
# speedup vs baseline: 1.0210x; 1.0009x over previous
"""Trainium2 Bass kernel for sparse channel-attention (XCA-style) module.

Reference computation (b=4, c=192, h=w=128, heads=6, C=32):
  qkv  = dwconv3x3(conv1x1(x, w_qkv), w_dw); ref_qkv likewise (shared weights)
  q = qkv[:, :c] (from x), k = ref_qkv[:, c:2c], v = ref_qkv[:, 2c:]
  q,k L2-normalized along tokens; attn = (q @ k^T) * temperature  [b,6,32,32]
  out = sum_i attn_w[i] * softmax(topk-threshold(attn, k_i)) @ v;  proj conv1x1.

Sharding: 8 cores = (batch 0..3) x (spatial half 0..1, 64 rows + halo).
Cross-core traffic: one 26KB AllReduce per core pair (q/k norms + q@k^T).

Device algorithm per core:
  - conv1x1 via float32r matmuls (1024-px double-buffered input granules;
    first block prefetched ahead of the weight loads)
  - dwconv3x3 q,k: chunks 0,1 on TensorE as 9 PSUM-accumulated
    diag-matmuls, chunk 2 on VectorE stt chains (engine balance)
  - dwconv3x3 v: v1 (64ch) packed two 4-row granules per K=128 block-diag
    matmul via a partition-duplicated, 4-row-shifted zv1 copy; v0 split
    PE-early / DVE-late / PE-collective-window to level the engines
  - PE transposes q,k to token-major; Gram per head-pair accumulates
    directly in PSUM across all 16 chunks (bank-aligned slices — two
    accumulation regions must never share a PSUM bank)
  - AllGather(pair) of stats in a 33-stride row layout (attn|qsq per row)
    so the consumer needs 3 DMAs + 3 adds; softmax with top-k via rank
    counting, all 4 branches batched into single wide DVE ops
  - final = (w_proj @ A_blockdiag) @ v with fo staged bf16 and out DMAd
    bf16 (host converts back to f32); ACT sqrt table preloaded at start
    so the tail pays only the exp-table load
"""

from contextlib import ExitStack

import numpy as np
import ml_dtypes

import concourse.bass as bass
import concourse.mybir as mybir
import concourse.tile as tile
from concourse import bacc
from concourse.bass_utils import run_bass_kernel_spmd

F32 = mybir.dt.float32
F32R = mybir.dt.float32r
BF16 = mybir.dt.bfloat16
AL = mybir.AluOpType
AF = mybir.ActivationFunctionType

B, CDIM, H, W = 4, 192, 128, 128
HEADS, CH = 6, 32
HB = 64                      # rows per core (half image)
ROWS = HB + 2                # halo rows in z buffer (66)
ZSTRIDE = 130                # padded row stride in z (128 + 2 zero pad cols)
ZBASE = 2                    # leading guard elements in z tiles
ZLEN = ZBASE + ROWS * ZSTRIDE + 2   # 8584
NPX = HB * W                 # output pixels per core (8192)
NIN = ROWS * W               # conv input pixels per core (8448)
KS_LIST = [16, 21, 24, 25]   # top-k values for C=32
# tap order: dw=0 taps first (even parity for DVE 2x mode)
TAPS9 = [(-1, 0), (0, 0), (1, 0), (-1, -1), (-1, 1), (0, -1), (0, 1), (1, -1), (1, 1)]

_CACHE = {}
import os
TRUNC = int(os.environ.get("KTRUNC", "9"))


def _build():
    nc = bacc.Bacc("TRN2", num_devices=8, num_swdge_queues=4)

    # ---------------- kernel I/O ----------------
    x_d = nc.dram_tensor("x_sh", [CDIM, NIN], F32R, kind="ExternalInput")
    r_d = nc.dram_tensor("ref_sh", [CDIM, NIN], F32R, kind="ExternalInput")
    wq_d = nc.dram_tensor("wq_t", [CDIM, 192], F32R, kind="ExternalInput")
    wkv_d = nc.dram_tensor("wkv_t", [CDIM, 384], F32R, kind="ExternalInput")
    dqk_d = nc.dram_tensor("dqk", [3, 128, 9 * 128], F32R, kind="ExternalInput")
    vw_d = nc.dram_tensor("vw", [CDIM, 9], F32, kind="ExternalInput")
    dv1_d = nc.dram_tensor("dv1", [128, 9 * 128], BF16, kind="ExternalInput")
    qkw_d = nc.dram_tensor("qkw", [3, 128, 9], F32, kind="ExternalInput")
    dv0_d = nc.dram_tensor("dv0", [128, 9 * 128], BF16, kind="ExternalInput")
    wp_d = nc.dram_tensor("wp_t", [CDIM, 192], BF16, kind="ExternalInput")
    temp_d = nc.dram_tensor("temp_rep", [CDIM, 1], F32, kind="ExternalInput")
    aw_d = nc.dram_tensor("aw_rep", [CDIM, 4], F32, kind="ExternalInput")
    ksr_d = nc.dram_tensor("ks_rep", [CDIM, 4], F32, kind="ExternalInput")
    idf_d = nc.dram_tensor("ident_f32", [128, 128], F32, kind="ExternalInput")
    e0_d = nc.dram_tensor("e_rep0", [HEADS, 128], F32, kind="ExternalInput")
    e1_d = nc.dram_tensor("e_rep1", [HEADS, 64], F32, kind="ExternalInput")
    out_d = nc.dram_tensor("out", [CDIM, NPX], BF16, kind="ExternalOutput")

    with tile.TileContext(nc) as tc, ExitStack() as ctx:
        consts = ctx.enter_context(tc.tile_pool(name="consts", bufs=1))
        zpool = ctx.enter_context(tc.tile_pool(name="zpool", bufs=1))
        zscp = ctx.enter_context(tc.tile_pool(name="zscp", bufs=2))
        ing = ctx.enter_context(tc.tile_pool(name="ing", bufs=2))    # input granules
        gcm = ctx.enter_context(tc.tile_pool(name="gcm", bufs=2))    # qk chan-major granules
        qktp = ctx.enter_context(tc.tile_pool(name="qktp", bufs=4))  # token-major qk tiles
        small = ctx.enter_context(tc.tile_pool(name="small", bufs=1))
        mps = ctx.enter_context(tc.tile_pool(name="mps", bufs=3, space="PSUM"))
        tps = ctx.enter_context(tc.tile_pool(name="tps", bufs=2, space="PSUM"))
        gaccp = ctx.enter_context(tc.tile_pool(name="gaccp", bufs=1, space="PSUM"))
        dram = ctx.enter_context(tc.tile_pool(name="dram", bufs=1, space="DRAM"))

        # ---------------- constant loads ----------------
        # prefetch the first 8-row input block BEFORE the weight loads so
        # the HWDGE delivers it first and the PE can start ASAP
        pref = {}
        for nm, dsrc, pw_ in (("xg0", x_d[0:128, 0:1024], 128),
                              ("xg1", x_d[128:192, 0:1024], 64),
                              ("rg0", r_d[0:128, 0:1024], 128),
                              ("rg1", r_d[128:192, 0:1024], 64)):
            t = ing.tile([pw_, 1024], F32R, tag=nm, name=nm)
            nc.sync.dma_start(t[:], dsrc)
            pref[nm] = t
        wq_sb0 = consts.tile([128, 192], F32R)
        wq_sb1 = consts.tile([64, 192], F32R)
        wkv_sb0 = consts.tile([128, 384], F32R)
        wkv_sb1 = consts.tile([64, 384], F32R)
        # weight loads ride the ACT-triggered queue so the input-granule
        # streaming DMAs (SP queue) reach the HWDGE first
        nc.scalar.dma_start(wq_sb0[:], wq_d[0:128, :])
        nc.scalar.dma_start(wq_sb1[:], wq_d[128:192, :])
        nc.scalar.dma_start(wkv_sb0[:], wkv_d[0:128, :])
        nc.scalar.dma_start(wkv_sb1[:], wkv_d[128:192, :])
        # group A: needed from the first pcc (dwconv + transposes)
        late_loads = []
        # group B: needed only from the collective window onward
        tail_loads = []
        dqk_sb = []
        for c in range(3):
            t = consts.tile([128, 9 * 128], F32R, name=f"dqk_sb{c}")
            late_loads.append((t, dqk_d[c]))
            dqk_sb.append(t)
        vw0 = consts.tile([128, 9], F32)
        vw1 = consts.tile([64, 9], F32)
        dv1_sb = consts.tile([128, 9 * 128], BF16)
        tail_loads.append((dv1_sb, dv1_d[:]))
        dv0_sb = consts.tile([128, 9 * 128], BF16)
        late_loads.append((dv0_sb, dv0_d[:]))
        qkw_sb = []
        for c in range(3):
            t = consts.tile([128, 9], F32, name=f"qkw_sb{c}")
            late_loads.append((t, qkw_d[c]))
            qkw_sb.append(t)
        late_loads.append((vw0, vw_d[0:128, :]))
        late_loads.append((vw1, vw_d[128:192, :]))
        wp0 = consts.tile([128, 192], BF16)
        wp1 = consts.tile([64, 192], BF16)
        tail_loads.append((wp0, wp_d[0:128, :]))
        tail_loads.append((wp1, wp_d[128:192, :]))
        temp0 = consts.tile([128, 1], F32)
        temp1 = consts.tile([64, 1], F32)
        tail_loads.append((temp0, temp_d[0:128, :]))
        tail_loads.append((temp1, temp_d[128:192, :]))
        aw0 = consts.tile([128, 4], F32)
        aw1 = consts.tile([64, 4], F32)
        tail_loads.append((aw0, aw_d[0:128, :]))
        tail_loads.append((aw1, aw_d[128:192, :]))
        ksr0 = consts.tile([128, 4], F32)
        ksr1 = consts.tile([64, 4], F32)
        tail_loads.append((ksr0, ksr_d[0:128, :]))
        tail_loads.append((ksr1, ksr_d[128:192, :]))
        ident_f32 = consts.tile([128, 128], F32)
        ident_r = consts.tile([128, 128], F32R)
        tail_loads.append((ident_f32, idf_d[:]))
        late_loads.append((ident_r, idf_d[:].bitcast(F32R)))
        erep0 = consts.tile([HEADS, 128], F32)
        erep1 = consts.tile([HEADS, 64], F32)
        tail_loads.append((erep0, e0_d[:]))
        tail_loads.append((erep1, e1_d[:]))

        # ---------------- z buffers ----------------
        # q,k conv outputs (z) kept in f32 (bf16 z flips top-k ranks and blows
        # the error budget), held as rolling 16-row super-chunks to fit SBUF.
        # v z-buffer stays full-size bf16 (v precision barely matters).
        SC_OUT = 16
        SC_IN = SC_OUT + 2
        ZSCLEN = ZBASE + SC_IN * ZSTRIDE + 2
        # zv1 is [128, ...]: partitions 0:64 hold v-channels 128:192 for z-row
        # r at slot r; partitions 64:128 hold the SAME channels for z-row r+4
        # at slot r (a DMA-duplicated, 4-row-shifted copy). This lets the v1
        # dwconv run as full K=128/M=128 block-diag matmuls covering two
        # 4-row granules at once.
        zv0 = zpool.tile([128, ZLEN], BF16)
        zv1 = zpool.tile([128, ZLEN], BF16)
        v0 = zpool.tile([128, NPX], BF16)
        # v1p: packed v1 output [128, NPX/2]: partitions 0:64 = 4-row groups
        # 0,2,4,..., partitions 64:128 = groups 1,3,5,...
        v1p = zpool.tile([128, NPX // 2], BF16)
        for zt in (zv0, zv1):
            nc.gpsimd.memset(zt[:, 0:ZBASE], 0.0)
            pad = zt[:, ZBASE:ZBASE + ROWS * ZSTRIDE].rearrange(
                "p (h w) -> p h w", w=ZSTRIDE)[:, :, 128:130]
            nc.gpsimd.memset(pad, 0.0)

        # Touch Sqrt once so ACT's initial function table is
        # "sqrt_and_friends" (which also holds Copy) — the tail's sqrt then
        # needs no table reload in the post-collective critical path.
        warm = small.tile([1, 2], F32, name="warm")
        nc.vector.memset(warm[:], 1.0)
        nc.scalar.sqrt(warm[:, 0:1], warm[:, 1:2])

        ncopy = [0]

        def copy_any(dst, src):
            # spread copy load: ACT takes 3 of 4 (DVE carries the c2 + v0
            # dwconv chains during the main phase)
            use_dve = (ncopy[0] % 4 == 0)
            ncopy[0] += 1
            if use_dve:
                nc.vector.tensor_copy(dst, src)
            else:
                nc.scalar.copy(dst, src)

        def zdst(zt, j0, nrows, p0, pw):
            # strided view of z rows j0..j0+nrows (cols 0..127)
            v = zt[p0:p0 + pw, ZBASE + ZSTRIDE * j0: ZBASE + ZSTRIDE * (j0 + nrows)]
            return v.rearrange("p (h w) -> p h w", w=ZSTRIDE)[:, :, 0:128]

        def ztap(zt, h0, nrows, dh, dw):
            # read view for output rows h0..h0+nrows, tap (dh, dw)
            start = ZBASE + ZSTRIDE * (h0 + 1 + dh) + dw
            v = zt[:, start:start + ZSTRIDE * nrows]
            return v.rearrange("p (h w) -> p h w", w=ZSTRIDE)[:, :, 0:128]

        # G accumulates directly in PSUM across all 16 pccs (64-matmul
        # accumulation chains). Each head-pair chunk's 256-wide slice gets
        # its OWN bank (512-stride) — two concurrent accumulation regions
        # sharing a bank corrupt each other.
        gacc = gaccp.tile([128, 1536], F32, name="gacc")
        PC_ROWS = 4

        def emit_v1_packed(h0g):
            # two 4-row granules (h0g, h0g+4) in one K=128 block-diag matmul
            # chain; partitions 64:128 of zv1 hold the 4-row-shifted dup.
            ps = tps.tile([128, 512], F32, tag="tp", name="v1_ps")
            for t, (dh, dw) in enumerate(TAPS9):
                nc.tensor.matmul(
                    ps[:, :].rearrange("p (h w) -> p h w", w=W),
                    lhsT=dv1_sb[:, t * 128:(t + 1) * 128],
                    rhs=ztap(zv1, h0g, PC_ROWS, dh, dw),
                    start=(t == 0), stop=(t == 8))
            g = h0g // 8
            copy_any(v1p[:, g * 512:(g + 1) * 512], ps[:, :])

        def emit_v0_dve(h0, nrows):
            # v channels 0:128 on DVE (16-row chunks amortize the op init;
            # gpsimd ucode has no TensorScalarPtr so Pool can't take these)
            outv = v0[:, h0 * W:(h0 + nrows) * W].rearrange(
                "p (h w) -> p h w", w=W)
            for t, (dh, dw) in enumerate(TAPS9):
                iv = ztap(zv0, h0, nrows, dh, dw)
                if t == 0:
                    nc.vector.tensor_scalar(
                        out=outv, in0=iv, scalar1=vw0[:, 0:1],
                        scalar2=None, op0=AL.mult)
                else:
                    nc.vector.scalar_tensor_tensor(
                        out=outv, in0=iv, scalar=vw0[:, t:t + 1],
                        in1=outv, op0=AL.mult, op1=AL.add)

        def emit_v0_pe(h0g):
            # 4-row granule on PE diag-matmuls (fills the collective window)
            ps = mps.tile([128, 512], F32, tag="main", name="v0_ps")
            for t, (dh, dw) in enumerate(TAPS9):
                nc.tensor.matmul(
                    ps[:, :].rearrange("p (h w) -> p h w", w=W),
                    lhsT=dv0_sb[:, t * 128:(t + 1) * 128],
                    rhs=ztap(zv0, h0g, PC_ROWS, dh, dw),
                    start=(t == 0), stop=(t == 8))
            copy_any(v0[:, h0g * W:(h0g + PC_ROWS) * W], ps[:, :])

        for sc in range(4):
            # --- conv1x1 (f32r) for this super-chunk: 18 input rows ---
            zsc = []
            for c in range(3):
                t_ = zscp.tile([128, ZSCLEN], F32R, tag=f"zsc{c}", name=f"zsc{c}")
                nc.vector.memset(t_[:, 0:ZBASE].bitcast(F32), 0.0)
                padv = t_[:, ZBASE:ZBASE + SC_IN * ZSTRIDE].rearrange(
                    "p (h w) -> p h w", w=ZSTRIDE)[:, :, 128:130].bitcast(F32)
                nc.vector.memset(padv, 0.0)
                zsc.append(t_)
            for (jd, drows) in ((0, 8), (8, 8), (16, 2)):
                nd = (SC_OUT * sc + jd) * W
                dpix = drows * W
                if jd == 0 and "xg0" in pref:
                    # first block: either the startup prefetch or the carry
                    # from the previous sc's boundary load
                    xg0, xg1 = pref.pop("xg0"), pref.pop("xg1")
                    rg0, rg1 = pref.pop("rg0"), pref.pop("rg1")
                elif jd == 16 and sc < 3:
                    # the 2-row tail IS the first 2 rows of the next sc's
                    # first block: load that block now and consume its head
                    for nm, src_d, pw_ in (("xg0", x_d, 128), ("xg1", x_d, 64),
                                           ("rg0", r_d, 128), ("rg1", r_d, 64)):
                        p0 = 0 if pw_ == 128 else 128
                        t = ing.tile([pw_, 1024], F32R, tag=nm, name=nm)
                        nc.sync.dma_start(
                            t[:], src_d[p0:p0 + pw_, nd:nd + 1024])
                        pref[nm] = t
                    xg0, xg1 = pref["xg0"], pref["xg1"]
                    rg0, rg1 = pref["rg0"], pref["rg1"]
                else:
                    xg0 = ing.tile([128, 1024], F32R, tag="xg0", name="xg0")
                    xg1 = ing.tile([64, 1024], F32R, tag="xg1", name="xg1")
                    rg0 = ing.tile([128, 1024], F32R, tag="rg0", name="rg0")
                    rg1 = ing.tile([64, 1024], F32R, tag="rg1", name="rg1")
                    nc.sync.dma_start(xg0[:, 0:dpix], x_d[0:128, nd:nd + dpix])
                    nc.sync.dma_start(xg1[:, 0:dpix], x_d[128:192, nd:nd + dpix])
                    nc.sync.dma_start(rg0[:, 0:dpix], r_d[0:128, nd:nd + dpix])
                    nc.sync.dma_start(rg1[:, 0:dpix], r_d[128:192, nd:nd + dpix])
                for js in range(0, drows, 4):
                    j0 = jd + js
                    nrows = min(4, drows - js)
                    npix = nrows * W
                    o0 = js * W
                    xrow = SC_OUT * sc + j0
                    for (co0, cow, zi, p0) in ((0, 128, 0, 0), (128, 64, 1, 0)):
                        ps = mps.tile([128, 512], F32, tag="main", name="cv_ps")
                        nc.tensor.matmul(ps[0:cow, 0:npix],
                                         lhsT=wq_sb0[:, co0:co0 + cow],
                                         rhs=xg0[:, o0:o0 + npix],
                                         start=True, stop=False)
                        nc.tensor.matmul(ps[0:cow, 0:npix],
                                         lhsT=wq_sb1[:, co0:co0 + cow],
                                         rhs=xg1[:, o0:o0 + npix],
                                         start=False, stop=True)
                        src = ps[0:cow, 0:npix].rearrange("p (h w) -> p h w", w=W)
                        copy_any(zdst(zsc[zi], j0, nrows, p0, cow), src)
                    kv_tiles = ((0, 64, ("sc", 1, 64)), (64, 128, ("sc", 2, 0)),
                                (192, 128, ("v", zv0, 0)), (320, 64, ("v", zv1, 0)))
                    for kvi, (co0, cow, dst) in enumerate(kv_tiles):
                        if kvi >= 2:
                            ps = tps.tile([128, 512], F32, tag="tp",
                                          name="cv_ps2")
                        else:
                            ps = mps.tile([128, 512], F32, tag="main",
                                          name="cv_ps")
                        nc.tensor.matmul(ps[0:cow, 0:npix],
                                         lhsT=wkv_sb0[:, co0:co0 + cow],
                                         rhs=rg0[:, o0:o0 + npix],
                                         start=True, stop=False)
                        nc.tensor.matmul(ps[0:cow, 0:npix],
                                         lhsT=wkv_sb1[:, co0:co0 + cow],
                                         rhs=rg1[:, o0:o0 + npix],
                                         start=False, stop=True)
                        src = ps[0:cow, 0:npix].rearrange("p (h w) -> p h w", w=W)
                        if dst[0] == "sc":
                            copy_any(zdst(zsc[dst[1]], j0, nrows, dst[2], cow), src)
                        else:
                            copy_any(zdst(dst[1], xrow, nrows, dst[2], cow), src)

            if sc == 0:
                for (tile_, dsrc) in late_loads:
                    nc.scalar.dma_start(tile_[:], dsrc)
            if sc == 2:
                for (tile_, dsrc) in tail_loads:
                    nc.scalar.dma_start(tile_[:], dsrc)
            # duplicate this sc's zv1 rows into partitions 64:128 shifted by
            # -4 rows (slot r holds z-row r+4) for the packed v1 matmuls
            r0d, r1d = (4, 18) if sc == 0 else (16 * sc + 2, 16 * sc + 18)
            nc.sync.dma_start(
                zv1[64:128, ZBASE + ZSTRIDE * (r0d - 4):ZBASE + ZSTRIDE * (r1d - 4)],
                zv1[0:64, ZBASE + ZSTRIDE * r0d:ZBASE + ZSTRIDE * r1d])
            # --- dwconv + transpose + Gram for output rows 16sc..16sc+16 ---
            for pcc in range(SC_OUT // PC_ROWS):
                h0l = pcc * PC_ROWS
                h0g = SC_OUT * sc + h0l
                grans = []
                pcg = sc * 4 + pcc
                for c in range(3):
                    g = gcm.tile([128, 512], F32R, tag=f"g{c}", name=f"gcm{c}", bufs=3 if c == 2 else 2)
                    if c == 2:
                        # DVE path: balances PE (the overall bottleneck)
                        gv = g[:].rearrange("p (h w) -> p h w", w=W)
                        for t, (dh, dw) in enumerate(TAPS9):
                            iv = ztap(zsc[c], h0l, PC_ROWS, dh, dw)
                            if t == 0:
                                nc.vector.tensor_scalar(
                                    out=gv, in0=iv, scalar1=qkw_sb[c][:, 0:1],
                                    scalar2=None, op0=AL.mult)
                            else:
                                nc.vector.scalar_tensor_tensor(
                                    out=gv, in0=iv, scalar=qkw_sb[c][:, t:t + 1],
                                    in1=gv, op0=AL.mult, op1=AL.add)
                    else:
                        ps = mps.tile([128, 512], F32, tag="main", name="dw_ps")
                        for t, (dh, dw) in enumerate(TAPS9):
                            nc.tensor.matmul(
                                ps[:, :].rearrange("p (h w) -> p h w", w=W),
                                lhsT=dqk_sb[c][:, t * 128:(t + 1) * 128],
                                rhs=ztap(zsc[c], h0l, PC_ROWS, dh, dw),
                                start=(t == 0), stop=(t == 8))
                        copy_any(g[:], ps[:])
                    grans.append(g)
                # v0 rows 0:48 ride along per-pcc: the first 6 chains on PE
                # (DVE is the bottleneck in sc0/sc1), the last 6 on DVE
                # during sc2/sc3 where DVE has slack. Rows 48:64 go to the
                # PE collective window.
                if pcg < 4:
                    emit_v0_pe(4 * pcg)
                elif 10 <= pcg < 14:
                    emit_v0_dve(16 + 4 * (pcg - 10), PC_ROWS)
                if TRUNC < 3:
                    continue
                # full 128x128 transposes only (sliced is_transpose crashes);
                # pair-grouping happens in the PSUM->SBUF copy via strided src
                qkts = []
                for r in range(PC_ROWS):
                    qkt_ps = tps.tile([128, 384], F32R, tag="tp", name="qkt_ps")
                    for c in range(3):
                        nc.tensor.transpose(
                            qkt_ps[:, 128 * c:128 * (c + 1)],
                            grans[c][:, 128 * r:128 * (r + 1)],
                            ident_r[:])
                    qkt = qktp.tile([128, 384], F32R, tag="qkt", name="qkt")
                    srcv = qkt_ps[:].rearrange(
                        "p (g hp c) -> p hp g c", g=2, hp=3)
                    copy_any(qkt[:, :].rearrange(
                        "p (hp g c) -> p hp g c", hp=3, g=2), srcv)
                    qkts.append(qkt)
                # full-width Gram (N=384 keeps f32r at 1 cyc/row); only the
                # diagonal 128-block per m-tile is accumulated
                for m in range(3):
                    # 256-wide moving window containing the pair block keeps
                    # f32r at 1 cyc/row (needs N >= 256); accumulate straight
                    # into the persistent PSUM gacc slice
                    w0 = 128 * m if m < 2 else 128
                    for r in range(PC_ROWS):
                        nc.tensor.matmul(
                            gacc[:, 512 * m:512 * m + 256],
                            lhsT=qkts[r][:, 128 * m:128 * (m + 1)],
                            rhs=qkts[r][:, w0:w0 + 256],
                            start=(pcg == 0 and r == 0),
                            stop=(pcg == 15 and r == PC_ROWS - 1))

        if TRUNC >= 4:
            # ---------------- phase 3: stats + AllReduce ----------------
            # stats layout (DMA/view-friendly, 33-stride rows):
            #   [0:4224)    128 rows x 33: q-rows 0:128 (heads 0-3): attn|qsq
            #   [4224:6336)  64 rows x 33: q-rows 128:192 (heads 4,5)
            #   [6336:6528) k-norms as (head, d)
            # AllReduce(add) over the core pair sums both spatial halves, so
            # the consumer needs no adds — just 3 loads with direct views.
            stats_in = dram.tile([6528], F32)
            stats_out = dram.tile([2, 6528], F32)
            nrm = small.tile([128, 3], F32)
            junk = small.tile([128, 128], F32)
            pstage = small.tile([32, 6 * CH], F32, name="pstage")
            for hp in range(3):
                goff = 512 * hp + (0 if hp < 2 else 128)
                gv = gacc[:, goff:goff + 128]
                nc.vector.tensor_tensor(out=junk[:], in0=gv,
                                        in1=ident_f32[:], op=AL.mult)
                nc.vector.tensor_reduce(out=nrm[:, hp:hp + 1], in_=junk[:],
                                        axis=mybir.AxisListType.X, op=AL.add)
                for j, (r0, c0) in enumerate(((0, 64), (32, 96))):
                    head = 2 * hp + j
                    nc.scalar.copy(pstage[:, CH * head:CH * (head + 1)],
                                   gv[r0:r0 + 32, c0:c0 + 32])
            # 3 producer DMAs: attn rows (all 6 heads, contiguous 33-stride),
            # qsq column (one strided view over all 3 pair-chunks), k-norms
            nc.sync.dma_start(
                stats_in[0:6336].rearrange("(h c j) -> c h j", h=6, j=33)[:, :, 0:32],
                pstage[:].rearrange("p (h d) -> p h d", d=CH))
            nc.sync.dma_start(
                stats_in[0:6336].rearrange("(c p j) -> p c j", c=3, j=33)[:, :, 32:33],
                nrm[0:64, :, None])
            nc.sync.dma_start(
                stats_in[6336:6528].rearrange("(hp p) -> p hp", hp=3),
                nrm[64:128, :])
            if TRUNC == 35:
                nc.sync.dma_start(stats_out[0], stats_in[:])
                nc.sync.dma_start(stats_out[1], stats_in[:])
            else:
                nc.gpsimd.collective_compute(
                    "AllGather", AL.bypass,
                    replica_groups=[[0, 1], [2, 3], [4, 5], [6, 7]],
                    ins=[stats_in[:].opt()], outs=[stats_out[:].opt()])
            # fill the collective window: packed v1 (8 matmul chains) and v0
            # rows 40:64 on PE (6 chains)
            for pb in range(8):
                emit_v1_packed(8 * pb)
                emit_v0_pe(32 + 4 * pb)

            if TRUNC >= 5 and TRUNC != 35:
                # ---------------- phase 4: softmax with top-k rank masks ----------------
                # one wide DMA per region pulls BOTH gathered halves; one add
                # each sums them in place
                sumA = small.tile([128, 66], F32, name="sumA")
                sumB = small.tile([64, 66], F32, name="sumB")
                ksql2 = small.tile([HEADS, 2 * CH], F32, name="ksql2")
                nc.sync.dma_start(
                    sumA[:].rearrange("p (g j) -> p g j", j=33),
                    stats_out[:, 0:4224].rearrange("g (p j) -> p g j", j=33))
                nc.sync.dma_start(
                    sumB[:].rearrange("p (g j) -> p g j", j=33),
                    stats_out[:, 4224:6336].rearrange("g (p j) -> p g j", j=33))
                nc.sync.dma_start(
                    ksql2[:].rearrange("h (g d) -> h g d", d=CH),
                    stats_out[:, 6336:6528].rearrange("g (h d) -> h g d", d=CH))
                nc.vector.tensor_tensor(out=sumA[:, 0:33], in0=sumA[:, 0:33],
                                        in1=sumA[:, 33:66], op=AL.add)
                nc.vector.tensor_tensor(out=sumB[:, 0:33], in0=sumB[:, 0:33],
                                        in1=sumB[:, 33:66], op=AL.add)
                ksql = ksql2[:, 0:CH]
                nc.vector.tensor_tensor(out=ksql, in0=ksql,
                                        in1=ksql2[:, CH:2 * CH], op=AL.add)
                attn = [sumA[:, 0:CH], sumB[:, 0:CH]]
                qsq = [sumA[:, CH:CH + 1], sumB[:, CH:CH + 1]]
                # replicate ksq across rows of each head via tiny matmul
                ksqr = [small.tile([128, CH], F32, name="ksqr0"),
                        small.tile([64, CH], F32, name="ksqr1")]
                for i, (erep, pw) in enumerate(((erep0, 128), (erep1, 64))):
                    ps = mps.tile([128, 512], F32, tag="main", name="krep_ps")
                    nc.tensor.matmul(ps[0:pw, 0:CH], lhsT=erep[:], rhs=ksql,
                                     start=True, stop=True)
                    nc.vector.tensor_copy(ksqr[i][:], ps[0:pw, 0:CH])

                abd0 = small.tile([128, 192], BF16)
                abd1 = small.tile([64, 192], BF16)
                nc.vector.memset(abd0[:], 0.0)
                nc.vector.memset(abd1[:], 0.0)

                # Two chains: ti=0 big ops on DVE, ti=1 big ops on gpsimd/Pool
                # (parallel); tiny scalar prep + reduces stay on DVE; ACT ops
                # grouped by function so the table loads (sqrt, exp) happen
                # once each.
                s_cs, iks, exs, rks, cmps = [], [], [], [], []
                for ti, pw in ((0, 128), (1, 64)):
                    tempt = temp0 if ti == 0 else temp1
                    s_c = small.tile([128, 1], F32, tag=f"s{ti}", name="s_c")
                    ik = small.tile([128, CH], F32, tag=f"ik{ti}", name="ik")
                    nc.scalar.sqrt(s_c[0:pw, :], qsq[ti])
                    nc.scalar.sqrt(ik[0:pw, :], ksqr[ti][:])
                    nc.vector.tensor_scalar(out=s_c[0:pw, :], in0=s_c[0:pw, :],
                                            scalar1=1e-12, scalar2=None, op0=AL.max)
                    nc.vector.reciprocal(s_c[0:pw, :], s_c[0:pw, :])
                    nc.vector.tensor_mul(s_c[0:pw, :], s_c[0:pw, :], tempt[:])
                    nc.vector.tensor_scalar(out=ik[0:pw, :], in0=ik[0:pw, :],
                                            scalar1=1e-12, scalar2=None, op0=AL.max)
                    nc.vector.reciprocal(ik[0:pw, :], ik[0:pw, :])
                    s_cs.append(s_c)
                    iks.append(ik)
                for ti, pw in ((0, 128), (1, 64)):
                    ve = nc.vector
                    at = attn[ti]
                    # attn = P * (temp/|q|) * (1/|k|); Pool ucode has no
                    # TensorScalarPtr, so ti=1 uses two tensor_tensor ops
                    nc.vector.scalar_tensor_tensor(
                        out=at, in0=at, scalar=s_cs[ti][0:pw, 0:1],
                        in1=iks[ti][0:pw, :], op0=AL.mult, op1=AL.mult)
                    cmp = small.tile([128, CH * CH], BF16, tag=f"cmp{ti}", name="cmp")
                    c3 = cmp[0:pw, :].rearrange("p (j k) -> p j k", k=CH)
                    ve.tensor_tensor(
                        out=c3, in0=at[:, None, :].broadcast_to([pw, CH, CH]),
                        in1=at[:, :, None].broadcast_to([pw, CH, CH]), op=AL.is_ge)
                    cmps.append(c3)
                for ti, pw in ((0, 128), (1, 64)):
                    rk = small.tile([128, CH], F32, tag=f"rk{ti}", name="rk")
                    nc.vector.tensor_reduce(out=rk[0:pw, :], in_=cmps[ti],
                                            axis=mybir.AxisListType.X, op=AL.add)
                    nmax = small.tile([128, 1], F32, tag=f"nm{ti}", name="nmax")
                    nc.vector.tensor_reduce(out=nmax[0:pw, :], in_=attn[ti],
                                            axis=mybir.AxisListType.X, op=AL.max,
                                            negate=True)
                    ex = small.tile([128, CH], F32, tag=f"ex{ti}", name="ex")
                    nc.scalar.activation(ex[0:pw, :], attn[ti], AF.Exp,
                                         bias=nmax[0:pw, 0:1], scale=1.0)
                    rks.append(rk)
                    exs.append(ex)
                for ti, pw in ((0, 128), (1, 64)):
                    ve = nc.vector
                    awt = aw0 if ti == 0 else aw1
                    ksrt = ksr0 if ti == 0 else ksr1
                    rk, ex = rks[ti], exs[ti]
                    # all 4 top-k branches batched: mk4[p, i, j] = [rk_j <= k_i]
                    mk4 = small.tile([128, 4 * CH], BF16, tag=f"mk{ti}", name="mk4")
                    m3 = mk4[0:pw, :].rearrange("p (i j) -> p i j", j=CH)
                    ve.tensor_tensor(
                        out=m3, in0=rk[0:pw, None, :].broadcast_to([pw, 4, CH]),
                        in1=ksrt[:, :, None].broadcast_to([pw, 4, CH]), op=AL.is_le)
                    junk4 = small.tile([128, 4 * CH], F32, tag=f"jk{ti}", name="junk4")
                    j3 = junk4[0:pw, :].rearrange("p (i j) -> p i j", j=CH)
                    ve.tensor_tensor(
                        out=j3, in0=m3,
                        in1=ex[0:pw, None, :].broadcast_to([pw, 4, CH]), op=AL.mult)
                    sden4 = small.tile([128, 4], F32, tag=f"sd{ti}", name="sden4")
                    nc.vector.tensor_reduce(out=sden4[0:pw, :], in_=j3,
                                            axis=mybir.AxisListType.X, op=AL.add)
                    coef4 = small.tile([128, 4], F32, tag=f"cf{ti}", name="coef4")
                    nc.vector.reciprocal(coef4[0:pw, :], sden4[0:pw, :])
                    nc.vector.tensor_mul(coef4[0:pw, :], coef4[0:pw, :],
                                         awt[:, 0:4])
                    # gt = sum_i coef4[:, i] * mk4[:, i, :] (stt is DVE-only)
                    gt = small.tile([128, CH], F32, tag=f"gt{ti}", name="gt")
                    for i in range(4):
                        if i == 0:
                            nc.vector.tensor_scalar(
                                out=gt[0:pw, :],
                                in0=mk4[0:pw, i * CH:(i + 1) * CH],
                                scalar1=coef4[0:pw, i:i + 1],
                                scalar2=None, op0=AL.mult)
                        else:
                            nc.vector.scalar_tensor_tensor(
                                out=gt[0:pw, :], in0=mk4[0:pw, i * CH:(i + 1) * CH],
                                scalar=coef4[0:pw, i:i + 1], in1=gt[0:pw, :],
                                op0=AL.mult, op1=AL.add)
                    # A blocks into block-diagonal abd (bf16)
                    abdt = abd0 if ti == 0 else abd1
                    nheads_t = 4 if ti == 0 else 2
                    for j in range(nheads_t):
                        head = j if ti == 0 else 4 + j
                        ve.tensor_tensor(
                            out=abdt[32 * j:32 * (j + 1), 32 * head:32 * (head + 1)],
                            in0=ex[32 * j:32 * (j + 1), :],
                            in1=gt[32 * j:32 * (j + 1), :], op=AL.mult)

                if TRUNC == 5:
                    nc.gpsimd.dma_start(out_d[0:128, 0:192], abd0[:])
                    nc.gpsimd.dma_start(out_d[128:192, 0:192], abd1[:])
                if TRUNC >= 9:
                    # ---------------- phase 5: M2^T = A_bd^T @ Wp^T, final = M2 @ v ----------------
                    # m2tb0: v0-part of M2^T; m2tb1: v1-part, duplicated into
                    # partitions 64:128 so odd-group v1p matmuls line up.
                    m2tb0 = small.tile([128, 192], BF16, name="m2tb0")
                    m2tb1 = small.tile([128, 192], BF16, name="m2tb1")
                    for dt_i, (d0, dw_) in enumerate(((0, 128), (128, 64))):
                        ps = mps.tile([128, 512], F32, tag="main", name="m2_ps")
                        nc.tensor.matmul(ps[0:dw_, 0:192], lhsT=abd0[:, d0:d0 + dw_],
                                         rhs=wp0[:], start=True, stop=False)
                        nc.tensor.matmul(ps[0:dw_, 0:192], lhsT=abd1[:, d0:d0 + dw_],
                                         rhs=wp1[:], start=False, stop=True)
                        if dt_i == 0:
                            nc.vector.tensor_copy(m2tb0[:], ps[0:128, 0:192])
                        else:
                            nc.vector.tensor_copy(m2tb1[0:64, :], ps[0:64, 0:192])
                            nc.vector.tensor_copy(m2tb1[64:128, :], ps[0:64, 0:192])

                    FCH = 512
                    for ct, (co0, cw) in enumerate(((0, 128), (128, 64))):
                        for fc in range(NPX // FCH):
                            n0 = fc * FCH
                            po = (fc % 2) * 64
            # rotate over 6 PSUM banks (main 4 + gp 2; gp pads to a full
                            # bank anyway) so copy-out latency doesn't
                            # throttle the matmuls
                            if fc % 2 == 1:
                                ps = tps.tile([128, 512], F32, tag="tp",
                                              name="fo_ps2")
                            else:
                                ps = mps.tile([128, 512], F32, tag="main",
                                              name="fo_ps")
                            nc.tensor.matmul(ps[0:cw, :], lhsT=m2tb0[:, co0:co0 + cw],
                                             rhs=v0[:, n0:n0 + FCH], start=True, stop=False)
                            nc.tensor.matmul(ps[0:cw, :],
                                             lhsT=m2tb1[po:po + 64, co0:co0 + cw],
                                             rhs=v1p[po:po + 64,
                                                     (fc // 2) * FCH:(fc // 2 + 1) * FCH],
                                             start=False, stop=True)
                            if fc % 2 == 0:
                                fo = ing.tile([128, 2 * FCH], BF16, tag="fo",
                                              name="fo_sb", bufs=3 if c == 2 else 2)
                            # alternate ACT/DVE: both are idle in the tail,
                            # and 32 serial ACT copies would gate the final
                            fdst = fo[0:cw, (fc % 2) * FCH:(fc % 2 + 1) * FCH]
                            if fc % 2 == 0:
                                nc.scalar.copy(fdst, ps[0:cw, :])
                            else:
                                nc.vector.tensor_copy(fdst, ps[0:cw, :])
                            if fc % 2 == 1:
                                nc.sync.dma_start(
                                    out_d[co0:co0 + cw, n0 - FCH:n0 + FCH],
                                    fo[0:cw, :])

    nc.finalize()
    return nc


def _prep_inputs(x, ref, w_qkv, w_dw, w_proj, temperature, attn_w):
    bf = ml_dtypes.bfloat16
    w_qkv = np.asarray(w_qkv, np.float32)[:, :, 0, 0]          # [576, 192]
    w_dw = np.asarray(w_dw, np.float32)[:, 0]                  # [576, 3, 3]
    w_proj = np.asarray(w_proj, np.float32)[:, :, 0, 0]        # [192, 192]
    temp = np.asarray(temperature, np.float32).reshape(HEADS)
    aw = np.asarray(attn_w, np.float32).reshape(4)

    wq_t = np.ascontiguousarray(w_qkv[:192].T)                 # [ci, co]
    wkv_t = np.ascontiguousarray(w_qkv[192:].T)                # [ci, 384]

    dwq, dwk, dwv = w_dw[:192], w_dw[192:384], w_dw[384:]
    chunks = [dwq[0:128], np.concatenate([dwq[128:192], dwk[0:64]]), dwk[64:192]]
    dqk = np.zeros((3, 128, 9 * 128), np.float32)
    for c, blk in enumerate(chunks):
        for t, (dh, dw_) in enumerate(TAPS9):
            np.fill_diagonal(dqk[c, :, t * 128:(t + 1) * 128], blk[:, dh + 1, dw_ + 1])
    qkw = np.zeros((3, 128, 9), np.float32)
    for c, blk in enumerate(chunks):
        for t, (dh, dw_) in enumerate(TAPS9):
            qkw[c, :, t] = blk[:, dh + 1, dw_ + 1]
    vw = np.zeros((CDIM, 9), np.float32)
    for t, (dh, dw_) in enumerate(TAPS9):
        vw[:, t] = dwv[:, dh + 1, dw_ + 1]
    dv1 = np.zeros((128, 9 * 128), np.float32)
    for t, (dh, dw_) in enumerate(TAPS9):
        np.fill_diagonal(dv1[0:64, t * 128:t * 128 + 64],
                         dwv[128:192, dh + 1, dw_ + 1])
        np.fill_diagonal(dv1[64:128, t * 128 + 64:(t + 1) * 128],
                         dwv[128:192, dh + 1, dw_ + 1])
    dv0 = np.zeros((128, 9 * 128), np.float32)
    for t, (dh, dw_) in enumerate(TAPS9):
        np.fill_diagonal(dv0[:, t * 128:(t + 1) * 128], dwv[0:128, dh + 1, dw_ + 1])

    wp_t = np.ascontiguousarray(w_proj.T)                      # [c, co]
    temp_rep = np.repeat(temp, CH).reshape(CDIM, 1)
    aw_rep = np.tile(aw[None, :], (CDIM, 1))
    ks_rep = np.tile(np.asarray(KS_LIST, np.float32)[None, :], (CDIM, 1))
    ident = np.eye(128, dtype=np.float32)
    e0 = np.zeros((HEADS, 128), np.float32)
    e1 = np.zeros((HEADS, 64), np.float32)
    for h in range(4):
        e0[h, 32 * h:32 * (h + 1)] = 1.0
    for h in range(2):
        e1[h + 4, 32 * h:32 * (h + 1)] = 1.0

    xp = np.zeros((B, CDIM, H + 2, W), np.float32)
    xp[:, :, 1:H + 1] = np.asarray(x, np.float32)
    rp = np.zeros((B, CDIM, H + 2, W), np.float32)
    rp[:, :, 1:H + 1] = np.asarray(ref, np.float32)

    common = {
        "wq_t": wq_t, "wkv_t": wkv_t,
        "dqk": dqk, "vw": vw, "qkw": qkw, "dv1": dv1.astype(bf), "dv0": dv0.astype(bf),
        "wp_t": wp_t.astype(bf), "temp_rep": temp_rep, "aw_rep": aw_rep,
        "ks_rep": ks_rep,
        "ident_f32": ident,
        "e_rep0": e0, "e_rep1": e1,
    }
    in_maps = []
    for core in range(8):
        b, s = core // 2, core % 2
        m = dict(common)
        m["x_sh"] = np.ascontiguousarray(
            xp[b, :, 64 * s:64 * s + ROWS].reshape(CDIM, NIN))
        m["ref_sh"] = np.ascontiguousarray(
            rp[b, :, 64 * s:64 * s + ROWS].reshape(CDIM, NIN))
        in_maps.append(m)
    return in_maps


def _run(inputs, trace=False):
    if "nc" not in _CACHE:
        _CACHE["nc"] = _build()
    nc = _CACHE["nc"]
    in_maps = _prep_inputs(**inputs)
    res = run_bass_kernel_spmd(nc, in_maps, core_ids=list(range(8)), trace=trace)
    out = np.zeros((B, CDIM, H, W), np.float32)
    for core in range(8):
        b, s = core // 2, core % 2
        out[b, :, 64 * s:64 * (s + 1)] = np.asarray(
            res.results[core]["out"], np.float32).reshape(CDIM, HB, W)
    return out, res


def kernel(**inputs):
    out, _ = _run(inputs, trace=False)
    return out

